# revision 1
# baseline (speedup 1.0000x reference)
"""GPT-NeoX attention layer (B=2, S=2048, E=2048, H=16, partial RoPE 32/128)
as a Bass/Tile kernel for 8 Trainium2 NeuronCores.

Sharding: tensor-parallel across heads (2 heads per core, Megatron-style).
Each core computes QKV projection for its 768 rows of w_qkv, applies partial
RoPE, runs causal attention for its 2 heads x 2 batches, and produces a
partial dense output (contraction over its 256 columns of w_dense).  The 8
partial outputs are summed on the host (no on-device collectives needed) and
the dense bias is added once on the host.

All device matmuls keep fp32 data in SBUF; the tensor engine runs them as
float32r (1 cycle/row for N>=256) with fp32 PSUM accumulation.

Layout choices (everything transposed once on the host so the contraction dim
always lands on SBUF partitions; no on-device transposes of x or weights):
  xT      [E, B*S]    x flattened and transposed
  wqkvT   [E, 768]    per-core slice of w_qkv, transposed
  wdT     [256, E]    per-core column-slice of w_dense, transposed
  qkvT    [768, B*S]  phase-1 output: per-head Q^T,K^T,V^T row blocks
  scores  S^T = (K^T)^T @ (Q^T) in [sk, sq] layout; softmax sums over the
          partition dim via a ones-matmul; y^T accumulated directly as
          V_nat^T @ P^T, which is the layout the dense matmul consumes.
"""

import numpy as np
from contextlib import ExitStack

import concourse.bass as bass
import concourse.bacc as bacc
import concourse.mybir as mybir
import concourse.tile as tile
from concourse.masks import make_identity

AF = mybir.ActivationFunctionType
F32 = mybir.dt.float32
F32R = mybir.dt.float32r

NEG_MASK = -1.0e9


class Cfg:
    def __init__(self, B=2, S=2048, E=2048, H=16, n_cores=8, mm_dtype=F32R):
        self.B, self.S, self.E, self.H = B, S, E, H
        self.HS = 128                 # head size (fixed: one partition tile)
        self.ROT = 32                 # rotary dims
        self.n_cores = n_cores
        self.HPC = H // n_cores       # heads per core
        self.R = 3 * self.HS * self.HPC   # per-core qkv rows
        self.SF = B * S               # flattened sequence
        self.KT = E // 128            # contraction tiles for qkv proj
        self.RT = self.R // 128       # row tiles of per-core qkv
        self.CW = self.HPC * self.HS  # per-core dense contraction width
        self.CT = self.CW // 128
        self.EO = E // 128            # dense output row tiles
        self.SCALE = 1.0 / np.sqrt(self.HS)
        self.mm_dtype = mm_dtype
        assert self.SF % 4 == 0 and S % 512 == 0 and E % 128 == 0


def build_program(cfg: Cfg) -> bass.Bass:
    B, S, E = cfg.B, cfg.S, cfg.E
    SF, R, KT, RT = cfg.SF, cfg.R, cfg.KT, cfg.RT
    HPC, ROT = cfg.HPC, cfg.ROT
    G = SF // 4                      # rope partition-regroup chunk
    mmdt = cfg.mm_dtype

    def rc(ap):
        # walrus requires every producer of an FP32r matmul operand to emit
        # FP32r (round-to-tf32); bitcast keeps the underlying tile fp32
        return ap.bitcast(mmdt) if mmdt == F32R else ap

    nc = bacc.Bacc(None)
    xT = nc.dram_tensor("xT", [E, SF], F32, kind="ExternalInput")
    wqkvT = nc.dram_tensor("wqkvT", [E, R], F32, kind="ExternalInput")
    bqkv = nc.dram_tensor("bqkv", [R], F32, kind="ExternalInput")
    wdT = nc.dram_tensor("wdT", [cfg.CW, E], F32, kind="ExternalInput")
    cos128 = nc.dram_tensor("cos128", [128, G], F32, kind="ExternalInput")
    sin128s = nc.dram_tensor("sin128s", [128, G], F32, kind="ExternalInput")
    maskT = nc.dram_tensor("maskT", [128, 128], F32, kind="ExternalInput")
    outT = nc.dram_tensor("outT", [E, SF], F32, kind="ExternalOutput")

    with tile.TileContext(nc) as tc, ExitStack() as stk:
        consts = stk.enter_context(tc.tile_pool(name="consts", bufs=1))
        qkvp = stk.enter_context(tc.tile_pool(name="qkvbuf", bufs=1))
        qkv_sb = qkvp.tile([128, RT, SF], F32)

        ident = consts.tile([128, 128], F32)
        make_identity(nc, ident)
        ones_k = consts.tile([128, 1], F32)    # lhsT for partition sums
        ones_m = consts.tile([1, 128], F32)    # lhsT for partition broadcast
        ones_tmp = consts.tile([128, 128], F32, tag="onestmp")
        nc.vector.memset(ones_tmp, 1.0)
        nc.vector.tensor_copy(rc(ones_k[:, :]), ones_tmp[:, 0:1])
        nc.vector.tensor_copy(rc(ones_m[:, :]), ones_tmp[0:1, :])
        mask_sb = consts.tile([128, 128], F32)
        nc.sync.dma_start(out=mask_sb, in_=maskT[:, :])
        bq_sb = consts.tile([128, RT], F32)
        nc.sync.dma_start(out=bq_sb, in_=bqkv.rearrange("(rt p) -> p rt", p=128))

        # ---------------- Phase 1: QKV projection -> qkvT in SBUF ----------
        SC = 256
        with tc.tile_pool(name="wq", bufs=1) as wp, \
             tc.tile_pool(name="xs", bufs=2) as xp, \
             tc.tile_pool(name="ps1", bufs=4, space="PSUM") as pp1:
            w_sb = wp.tile([128, KT, R], F32)
            nc.sync.dma_start(
                out=rc(w_sb[:, :, :]),
                in_=rc(wqkvT.rearrange("(kt p) r -> p kt r", p=128)))
            x_view = xT.rearrange("(kt p) s -> p kt s", p=128)
            for sc in range(SF // SC):
                xt = xp.tile([128, KT, SC], F32)
                nc.sync.dma_start(out=rc(xt[:, :, :]),
                                  in_=rc(x_view[:, :, sc * SC:(sc + 1) * SC]))
                for rt in range(RT):
                    ps = pp1.tile([128, SC], F32)
                    for kt in range(KT):
                        nc.tensor.matmul(
                            ps,
                            w_sb[:, kt, rt * 128:(rt + 1) * 128].bitcast(mmdt),
                            xt[:, kt, :].bitcast(mmdt),
                            start=(kt == 0), stop=(kt == KT - 1))
                    # evict + qkv bias (per-partition) on ScalarE
                    nc.scalar.activation(
                        rc(qkv_sb[:, rt, sc * SC:(sc + 1) * SC]), ps,
                        AF.Identity, bias=bq_sb[:, rt:rt + 1])

        # ---------------- RoPE on first ROT rows of each Q^T / K^T ---------
        # Regroup [ROT, SF] -> [128, SF/4] so the DVE ops use all lanes.
        with tc.tile_pool(name="rope", bufs=2) as rp:
            cos_sb = rp.tile([128, G], F32, tag="costab")
            sin_sb = rp.tile([128, G], F32, tag="sintab")
            nc.sync.dma_start(out=cos_sb, in_=cos128[:, :])
            nc.sync.dma_start(out=sin_sb, in_=sin128s[:, :])
            half = ROT // 2
            for h in range(HPC):
                for qk in range(2):
                    rt = 3 * h + qk
                    blk = qkv_sb[0:ROT, rt, :]
                    plain = rp.tile([128, G], F32, tag="plain")
                    sw = rp.tile([128, G], F32, tag="swap")
                    for g in range(4):
                        gs = slice(g * G, (g + 1) * G)
                        nc.sync.dma_start(
                            out=plain[g * 32:(g + 1) * 32, :], in_=blk[:, gs])
                        # rotate_half: rows 0:16 <- rows 16:32, rows 16:32 <- 0:16
                        nc.sync.dma_start(
                            out=sw[g * 32:g * 32 + half, :],
                            in_=qkv_sb[half:ROT, rt, gs])
                        nc.sync.dma_start(
                            out=sw[g * 32 + half:(g + 1) * 32, :],
                            in_=qkv_sb[0:half, rt, gs])
                    nc.vector.tensor_mul(plain, plain, cos_sb)
                    nc.vector.tensor_mul(sw, sw, sin_sb)   # sign folded in table
                    nc.vector.tensor_add(rc(plain[:, :]), plain, sw)
                    for g in range(4):
                        gs = slice(g * G, (g + 1) * G)
                        nc.sync.dma_start(
                            out=rc(qkv_sb[0:ROT, rt, gs]),
                            in_=rc(plain[g * 32:(g + 1) * 32, :]))

        # ---------------- Phase 2+3: attention + partial dense -------------
        NCH = S // 512                    # sq chunks per (b, h) pair
        with tc.tile_pool(name="yt", bufs=1) as yp, \
             tc.tile_pool(name="wd", bufs=1) as wdp, \
             tc.tile_pool(name="vnat", bufs=2) as vp, \
             tc.tile_pool(name="pstrip", bufs=3) as ppool, \
             tc.tile_pool(name="norm", bufs=2) as npool, \
             tc.tile_pool(name="outsb", bufs=4) as op, \
             tc.tile_pool(name="psA", bufs=2, space="PSUM") as psA, \
             tc.tile_pool(name="psY", bufs=2, space="PSUM") as psY, \
             tc.tile_pool(name="psS", bufs=2, space="PSUM") as psS, \
             tc.tile_pool(name="psD", bufs=2, space="PSUM") as psD:
            yT_sb = yp.tile([128, HPC, SF], F32)
            wd_sb = wdp.tile([128, cfg.CT, E], F32)
            nc.sync.dma_start(
                out=rc(wd_sb[:, :, :]),
                in_=rc(wdT.rearrange("(ct p) e -> p ct e", p=128)))

            def dense_cols(b):
                # partial dense for columns of batch b, overlaps next pair
                sc0 = b * (S // 512)
                for eo in range(cfg.EO):
                    for scn in range(S // 512):
                        col = b * S + scn * 512
                        ps = psD.tile([128, 512], F32)
                        for ct in range(cfg.CT):
                            nc.tensor.matmul(
                                ps,
                                wd_sb[:, ct, eo * 128:(eo + 1) * 128].bitcast(mmdt),
                                yT_sb[:, ct, col:col + 512].bitcast(mmdt),
                                start=(ct == 0), stop=(ct == cfg.CT - 1))
                        ot = op.tile([128, 512], F32)
                        if (eo + scn) % 2 == 0:
                            nc.vector.tensor_copy(ot, ps)
                        else:
                            nc.scalar.activation(ot, ps, AF.Copy)
                        nc.sync.dma_start(
                            out=outT[eo * 128:(eo + 1) * 128, col:col + 512],
                            in_=ot)

            for b in range(B):
                for h in range(HPC):
                    scol = b * S
                    q_t = qkv_sb[:, 3 * h + 0, scol:scol + S]
                    k_t = qkv_sb[:, 3 * h + 1, scol:scol + S]
                    v_t = qkv_sb[:, 3 * h + 2, scol:scol + S]
                    njt = S // 128
                    # V natural layout [sk, d] via PE transpose of V^T tiles
                    vnat = vp.tile([128, njt, 128], F32)
                    for jt in range(njt):
                        pst = psA.tile([128, 512], F32, tag="A")
                        nc.tensor.transpose(
                            pst[:, 0:128], v_t[:, jt * 128:(jt + 1) * 128], ident)
                        nc.vector.tensor_copy(rc(vnat[:, jt, :]), pst[:, 0:128])
                    for c in range(NCH):
                        yacc = psY.tile([128, 512], F32)
                        sums = psS.tile([1, 512], F32)
                        nj = 4 * c + 4
                        for j in range(nj):
                            off = max(0, j * 128 - c * 512)
                            n = 512 - off
                            first, last = (j == 0), (j == nj - 1)
                            ps = psA.tile([128, 512], F32, tag="A")
                            nc.tensor.matmul(
                                ps[:, off:],
                                k_t[:, j * 128:(j + 1) * 128].bitcast(mmdt),
                                q_t[:, c * 512 + off:c * 512 + 512].bitcast(mmdt),
                                start=True, stop=True, skip_group_check=True)
                            if j >= 4 * c:  # diagonal block: causal mask
                                nc.vector.tensor_add(
                                    ps[:, off:off + 128], ps[:, off:off + 128],
                                    mask_sb)
                            pT = ppool.tile([128, 512], F32)
                            nc.scalar.activation(
                                rc(pT[:, off:]), ps[:, off:], AF.Exp,
                                scale=cfg.SCALE)
                            nc.tensor.matmul(
                                sums[:, off:], ones_k.bitcast(mmdt),
                                pT[:, off:].bitcast(mmdt),
                                start=first, stop=last, skip_group_check=True)
                            nc.tensor.matmul(
                                yacc[:, off:], vnat[:, j, :].bitcast(mmdt),
                                pT[:, off:].bitcast(mmdt),
                                start=first, stop=last, skip_group_check=True)
                        # normalize: recip of sums, broadcast over partitions
                        recip = npool.tile([1, 512], F32, tag="recip")
                        with nc.allow_low_precision(
                                reason="tf32 rounding for fp32r matmul"):
                            nc.vector.reciprocal(rc(recip[:, :]), sums)
                        bc = psA.tile([128, 512], F32, tag="A")
                        nc.tensor.matmul(
                            bc, ones_m.bitcast(mmdt), recip.bitcast(mmdt),
                            start=True, stop=True, skip_group_check=True)
                        bcs = npool.tile([128, 512], F32, tag="bcs")
                        nc.vector.tensor_copy(bcs, bc)
                        nc.vector.tensor_mul(
                            rc(yT_sb[:, h, scol + c * 512:scol + (c + 1) * 512]),
                            yacc, bcs)
                dense_cols(b)

    nc.finalize()
    return nc


# ---------------------------------------------------------------------------
# Host-side input preparation / sharding
# ---------------------------------------------------------------------------

def _tf32_round(a: np.ndarray) -> np.ndarray:
    """Round fp32 to tf32 (round-to-nearest-even on the low 13 mantissa bits).
    DMA cannot round, so FP32r matmul operands fed straight from DRAM are
    pre-rounded on the host."""
    u = np.ascontiguousarray(a, np.float32).view(np.uint32)
    u = (u + 0x0FFF + ((u >> 13) & 1)) & np.uint32(0xFFFFE000)
    return u.view(np.float32)


def _rope_tables(cfg: Cfg):
    inv_freq = 1.0 / (10000.0 ** (np.arange(0, cfg.ROT, 2, dtype=np.float64)
                                  / cfg.ROT))
    t = np.arange(cfg.S, dtype=np.float64)
    freqs = np.outer(t, inv_freq)                       # [S, 16]
    emb = np.concatenate([freqs, freqs], axis=-1)       # [S, 32]
    cos = np.cos(emb).T.astype(np.float32)              # [32, S]
    sin = np.sin(emb).T.astype(np.float32)
    cosF = np.tile(cos, (1, cfg.B))                     # [32, SF]
    sinF = np.tile(sin, (1, cfg.B))
    sinF[:cfg.ROT // 2] *= -1.0                         # fold rotate_half sign
    G = cfg.SF // 4
    cos128 = np.ascontiguousarray(
        cosF.reshape(32, 4, G).transpose(1, 0, 2).reshape(128, G))
    sin128s = np.ascontiguousarray(
        sinF.reshape(32, 4, G).transpose(1, 0, 2).reshape(128, G))
    return cos128, sin128s


def make_in_maps(cfg: Cfg, x, w_qkv, b_qkv, w_dense):
    rnd = _tf32_round if cfg.mm_dtype == F32R else (
        lambda a: np.ascontiguousarray(a, np.float32))
    xT = rnd(x.reshape(cfg.B * cfg.S, cfg.E).T)
    cos128, sin128s = _rope_tables(cfg)
    p = np.arange(128)[:, None]
    f = np.arange(128)[None, :]
    maskT = np.where(p <= f, 0.0, NEG_MASK).astype(np.float32)
    in_maps = []
    for i in range(cfg.n_cores):
        rows = slice(i * cfg.R, (i + 1) * cfg.R)
        cols = slice(i * cfg.CW, (i + 1) * cfg.CW)
        in_maps.append({
            "xT": xT,
            "wqkvT": rnd(w_qkv[rows, :].T),
            "bqkv": np.ascontiguousarray(b_qkv[rows]).astype(np.float32),
            "wdT": rnd(w_dense[:, cols].T),
            "cos128": cos128,
            "sin128s": sin128s,
            "maskT": maskT,
        })
    return in_maps


def combine_outputs(cfg: Cfg, results, b_dense):
    acc = np.zeros((cfg.E, cfg.SF), dtype=np.float64)
    for r in results:
        acc += r["outT"].astype(np.float64)
    out = acc.T.reshape(cfg.B, cfg.S, cfg.E) + b_dense.astype(np.float64)
    return out.astype(np.float32)


_PROGRAM_CACHE = {}


def kernel(x, w_qkv, b_qkv, w_dense, b_dense):
    from concourse.bass_utils import run_bass_kernel_spmd

    cfg = Cfg()
    key = "full"
    if key not in _PROGRAM_CACHE:
        _PROGRAM_CACHE[key] = build_program(cfg)
    nc = _PROGRAM_CACHE[key]
    in_maps = make_in_maps(cfg, np.asarray(x), np.asarray(w_qkv),
                           np.asarray(b_qkv), np.asarray(w_dense))
    res = run_bass_kernel_spmd(nc, in_maps, list(range(cfg.n_cores)))
    return combine_outputs(cfg, res.results, np.asarray(b_dense))



# revision 13
# speedup vs baseline: 1.3456x; 1.3456x over previous
"""GPT-NeoX attention layer (B=2, S=2048, E=2048, H=16, partial RoPE 32/128)
as a Bass/Tile kernel for 8 Trainium2 NeuronCores.

Sharding: tensor-parallel across heads (2 heads per core, Megatron-style).
Each core computes QKV projection for its 2 heads over all tokens, applies
partial RoPE, runs causal attention, and produces a partial dense output
(contraction over its 256 columns of w_dense).  The 8 bf16 partial outputs
are summed on the host and the dense bias is added once on the host.

Everything on-device is bf16 (inputs pre-converted on the host); PSUM
accumulation stays fp32.  Key structure choices:

  qk_sb  [128, 4, SF]   Q^T/K^T per head (head dim on partitions) - scores
                        and y^T matmuls consume this directly.
  vnat   [128, SF/128, 256]  V in NATURAL [token, d] layout, produced in
                        phase 1 by x-stationary matmuls (x as lhsT), so no
                        PE transposes of V are ever needed.
  scores S^T = K^T.T @ Q^T in [sk, sq] blocks; exp on ScalarE (pipelined one
                        block behind the scores matmuls).
  softmax sums          via N=1 matmuls with the exp'd block as the
                        stationary operand (out [sq,1]): nearly free on PE,
                        instead of a 512-wide ones-matmul per block.
  normalize             reciprocal -> tiny PE transpose -> GPSIMD
                        partition_broadcast -> one DVE multiply per chunk.
  dense                 interleaved into later attention heads (fills the
                        tensor engine while ScalarE works through exp).
"""

import numpy as np
from contextlib import ExitStack

import concourse.bass as bass
import concourse.bacc as bacc
import concourse.mybir as mybir
import concourse.tile as tile
from concourse.masks import make_identity

AF = mybir.ActivationFunctionType
F32 = mybir.dt.float32
BF16 = mybir.dt.bfloat16

NEG_MASK = -1.0e9


class Cfg:
    def __init__(self, B=2, S=2048, E=2048, H=16, n_cores=8):
        self.B, self.S, self.E, self.H = B, S, E, H
        self.HS = 128                  # head size (one partition tile)
        self.ROT = 32                  # rotary dims
        self.n_cores = n_cores
        self.HPC = H // n_cores        # heads per core
        assert self.HPC == 2, "kernel assumes 2 heads per core"
        self.NQK = 2 * self.HPC        # q/k row tiles (h0q,h0k,h1q,h1k)
        self.VW = self.HPC * self.HS   # v natural width (d per core)
        self.RW = self.NQK * self.HS   # per-core q+k rows
        self.WCOLS = self.RW + self.VW
        self.SF = B * S
        self.KT = E // 128             # contraction tiles
        self.SC = 256                  # phase-1 token chunk
        self.NP1 = self.SF // self.SC
        self.G = self.SF // 4          # rope regroup width
        self.NQC = S // 512            # q chunks per (b, h)
        self.EO = E // 128             # dense output row tiles
        self.CT = self.HPC             # dense contraction tiles
        self.SCALE = 1.0 / np.sqrt(self.HS)
        assert S % 512 == 0 and E % 128 == 0 and self.SF % (4 * self.SC) == 0


class _Feeder:
    """Round-robin sink of deferred emission micro-steps (dense tiles)."""

    def __init__(self):
        self.gens = []

    def push(self, gen):
        self.gens.append(gen)

    def step(self):
        while self.gens:
            try:
                next(self.gens[0])
                return
            except StopIteration:
                self.gens.pop(0)

    def drain(self):
        while self.gens:
            g = self.gens.pop(0)
            for _ in g:
                pass


def build_program(cfg: Cfg, debug: bool = False) -> bass.Bass:
    B, S, E = cfg.B, cfg.S, cfg.E
    SF, KT, G = cfg.SF, cfg.KT, cfg.G
    SC, NQK, VW, RW = cfg.SC, cfg.NQK, cfg.VW, cfg.RW
    HPC, CT, EO = cfg.HPC, cfg.CT, cfg.EO
    NT = SF // 128                   # vnat token tiles

    nc = bacc.Bacc(None)
    xT = nc.dram_tensor("xT", [E, SF], BF16, kind="ExternalInput")
    wcat = nc.dram_tensor("wcat", [E, cfg.WCOLS], BF16, kind="ExternalInput")
    bqk = nc.dram_tensor("bqk", [RW], F32, kind="ExternalInput")
    vbbc = nc.dram_tensor("vbbc", [128, VW], F32, kind="ExternalInput")
    wdT = nc.dram_tensor("wdT", [VW, E], BF16, kind="ExternalInput")
    cosT = nc.dram_tensor("cosT", [128, G], BF16, kind="ExternalInput")
    sinT = nc.dram_tensor("sinT", [128, G], BF16, kind="ExternalInput")
    maskT = nc.dram_tensor("maskT", [128, 128], F32, kind="ExternalInput")
    outT = nc.dram_tensor("outT", [E, SF], BF16, kind="ExternalOutput")

    with tile.TileContext(nc) as tc, ExitStack() as stk:
        consts = stk.enter_context(tc.tile_pool(name="consts", bufs=1))
        bigp = stk.enter_context(tc.tile_pool(name="big", bufs=1))
        qk_sb = bigp.tile([128, NQK, SF], BF16)
        vnat = bigp.tile([128, NT, VW], BF16)
        yT_sb = bigp.tile([128, CT, SF], BF16)

        # ---------------- Phase 1: QKV projection ------------------------
        with tc.tile_pool(name="wq", bufs=1) as wp, \
             tc.tile_pool(name="xs", bufs=2) as xp, \
             tc.tile_pool(name="ps1", bufs=2, space="PSUM") as pp1:
            w_sb = wp.tile([128, KT, cfg.WCOLS], BF16)
            w_view = wcat.rearrange("(kt p) r -> p kt r", p=128)
            x_view = xT.rearrange("(kt p) s -> p kt s", p=128)

            # interleave per-kt w loads with quarters of the first x chunk
            xt0 = xp.tile([128, KT, SC], BF16, tag="xt")
            ktg = max(1, KT // 4)
            for q0 in range(0, KT, ktg):
                q1 = min(q0 + ktg, KT)
                for kt in range(q0, q1):
                    nc.sync.dma_start(out=w_sb[:, kt, :], in_=w_view[:, kt, :])
                nc.sync.dma_start(out=xt0[:, q0:q1, :],
                                  in_=x_view[:, q0:q1, 0:SC])
            xt1 = xp.tile([128, KT, SC], BF16, tag="xt")
            nc.sync.dma_start(out=xt1[:, :, :], in_=x_view[:, :, SC:2 * SC])

            # constants (after the critical w/x stream)
            ident = consts.tile([128, 128], F32)
            make_identity(nc, ident)
            identB = consts.tile([128, 128], BF16)
            with nc.allow_low_precision(reason="bf16 identity"):
                nc.vector.tensor_copy(identB, ident)
            ones_tmp = consts.tile([128, 1], BF16, tag="onestmp")
            nc.vector.memset(ones_tmp, 1.0)
            ones_col = ones_tmp
            mask_sb = consts.tile([128, 128], F32)
            nc.sync.dma_start(out=mask_sb, in_=maskT[:, :])
            bqk_sb = consts.tile([128, NQK], F32)
            nc.sync.dma_start(out=bqk_sb,
                              in_=bqk.rearrange("(rt p) -> p rt", p=128))
            vb_sb = consts.tile([128, VW], F32)
            nc.sync.dma_start(out=vb_sb, in_=vbbc[:, :])
            cos_sb = consts.tile([128, G], BF16, tag="costab")
            sin_sb = consts.tile([128, G], BF16, tag="sintab")
            nc.sync.dma_start(out=cos_sb, in_=cosT[:, :])
            nc.sync.dma_start(out=sin_sb, in_=sinT[:, :])
            wd_sb = consts.tile([128, CT, E], BF16, tag="wd")
            nc.sync.dma_start(
                out=wd_sb[:, :, :],
                in_=wdT.rearrange("(ct p) e -> p ct e", p=128))

            ntile = SC // 128   # v token sub-tiles per chunk (=2)
            for sc in range(cfg.NP1):
                if sc == 0:
                    xt = xt0
                elif sc == 1:
                    xt = xt1
                else:
                    xt = xp.tile([128, KT, SC], BF16, tag="xt")
                    nc.sync.dma_start(
                        out=xt[:, :, :],
                        in_=x_view[:, :, sc * SC:(sc + 1) * SC])
                qkps = [pp1.tile([128, 512], F32, tag=f"qk{i}", name=f"qkps{i}")
                        for i in range(NQK // 2)]
                vps = pp1.tile([128, 512], F32, tag="v")
                # NOTE: a start=True matmul marks the PSUM bank's whole 2KB
                # zero-region pending-zero, so only the FIRST matmul into
                # each bank starts; co-resident column groups accumulate
                # onto pending-zero bytes.
                for kt in range(KT):
                    fl, ll = (kt == 0), (kt == KT - 1)
                    for i in range(NQK):
                        nc.tensor.matmul(
                            qkps[i // 2][:, 256 * (i % 2):256 * (i % 2) + SC],
                            w_sb[:, kt, 128 * i:128 * (i + 1)],
                            xt[:, kt, :],
                            start=fl and i % 2 == 0,
                            stop=ll and i % 2 == 1, skip_group_check=True)
                    for t in range(ntile):
                        nc.tensor.matmul(
                            vps[:, VW * t:VW * (t + 1)],
                            xt[:, kt, 128 * t:128 * (t + 1)],
                            w_sb[:, kt, RW:RW + VW],
                            start=fl and t == 0,
                            stop=ll and t == ntile - 1, skip_group_check=True)
                for i in range(NQK):
                    nc.scalar.activation(
                        qk_sb[:, i, sc * SC:(sc + 1) * SC],
                        qkps[i // 2][:, 256 * (i % 2):256 * (i % 2) + SC],
                        AF.Identity, bias=bqk_sb[:, i:i + 1])
                with nc.allow_low_precision(reason="bf16 v eviction"):
                    for t in range(ntile):
                        nc.vector.tensor_add(
                            vnat[:, sc * ntile + t, :],
                            vps[:, VW * t:VW * (t + 1)], vb_sb)

        # ---------------- RoPE on first 32 rows of each Q^T/K^T ----------
        half = cfg.ROT // 2
        with tc.tile_pool(name="rope", bufs=2) as rp:
            for i in range(NQK):
                blk = qk_sb[0:cfg.ROT, i, :]
                plain = rp.tile([128, G], BF16, tag="plain")
                sw = rp.tile([128, G], BF16, tag="swap")
                for g in range(4):
                    gs = slice(g * G, (g + 1) * G)
                    nc.sync.dma_start(
                        out=plain[g * 32:(g + 1) * 32, :], in_=blk[:, gs])
                    nc.sync.dma_start(
                        out=sw[g * 32:g * 32 + half, :],
                        in_=qk_sb[half:cfg.ROT, i, gs])
                    nc.sync.dma_start(
                        out=sw[g * 32 + half:(g + 1) * 32, :],
                        in_=qk_sb[0:half, i, gs])
                with nc.allow_low_precision(reason="bf16 rope"):
                    nc.vector.tensor_mul(plain, plain, cos_sb)
                    nc.vector.tensor_mul(sw, sw, sin_sb)  # sign folded in
                    nc.vector.tensor_add(plain, plain, sw)
                for g in range(4):
                    gs = slice(g * G, (g + 1) * G)
                    nc.sync.dma_start(
                        out=qk_sb[0:cfg.ROT, i, gs],
                        in_=plain[g * 32:(g + 1) * 32, :])

        # ---------------- Attention + interleaved dense -------------------
        feeder = _Feeder()
        with tc.tile_pool(name="pstrip", bufs=3) as ptp, \
             tc.tile_pool(name="norm", bufs=2) as npool, \
             tc.tile_pool(name="outsb", bufs=4) as op, \
             tc.tile_pool(name="psA", bufs=3, space="PSUM") as psA, \
             tc.tile_pool(name="psY", bufs=2, space="PSUM") as psY, \
             tc.tile_pool(name="psS", bufs=1, space="PSUM") as psS, \
             tc.tile_pool(name="psD", bufs=2, space="PSUM") as psD:

            def dense_steps(b, scp):
                # one (eo) output row-tile over two 512-token col chunks
                for eo in range(EO):
                    ot = op.tile([128, 1024], BF16, tag="out")
                    for t in range(2):
                        col = b * S + (2 * scp + t) * 512
                        pd = psD.tile([128, 512], F32, tag="D")
                        for ct in range(CT):
                            nc.tensor.matmul(
                                pd,
                                wd_sb[:, ct, 128 * eo:128 * (eo + 1)],
                                yT_sb[:, ct, col:col + 512],
                                start=(ct == 0), stop=(ct == CT - 1),
                                skip_group_check=True)
                        yield
                        with nc.allow_low_precision(reason="bf16 out"):
                            if (eo + t) % 2 == 0:
                                nc.vector.tensor_copy(
                                    ot[:, 512 * t:512 * (t + 1)], pd)
                            else:
                                nc.scalar.activation(
                                    ot[:, 512 * t:512 * (t + 1)], pd, AF.Copy)
                    nc.sync.dma_start(
                        out=outT[128 * eo:128 * (eo + 1),
                                 b * S + scp * 1024:b * S + (scp + 1) * 1024],
                        in_=ot)
                    yield

            def attention(b, hl):
                scol = b * S
                q_t = qk_sb[:, 2 * hl, scol:scol + S]
                k_t = qk_sb[:, 2 * hl + 1, scol:scol + S]
                pending = [None]   # chunk-end normalization closure

                def emit_chain(c, psYt, psSt):
                    recip = npool.tile([128, 4], F32, tag="recip")
                    nc.vector.reciprocal(recip, psSt[:, 0:4])
                    # transpose each recip column to partition 0 ([1, 128])
                    rps = psA.tile([128, 512], F32, tag="A", name="rps")
                    for g in range(4):
                        nc.tensor.matmul(
                            rps[0:1, 128 * g:128 * (g + 1)],
                            recip[:, g:g + 1], ident,
                            is_transpose=True, start=(g == 0), stop=(g == 3),
                            skip_group_check=True)
                    rT = npool.tile([1, 512], F32, tag="rT")
                    nc.vector.tensor_copy(rT, rps[0:1, 0:512])
                    bc = npool.tile([128, 512], F32, tag="bc")
                    for g in range(4):
                        nc.gpsimd.partition_broadcast(
                            bc[:, 128 * g:128 * (g + 1)],
                            rT[0:1, 128 * g:128 * (g + 1)])
                    with nc.allow_low_precision(reason="bf16 y eviction"):
                        nc.vector.tensor_mul(
                            yT_sb[:, hl, scol + c * 512:scol + (c + 1) * 512],
                            psYt[:, 0:512], bc)

                for c in range(cfg.NQC):
                    nj = 4 * (c + 1)
                    psYt = psY.tile([128, 512], F32, tag="Y")
                    psSt = psS.tile([128, 4], F32, tag="S")
                    prev = None

                    def emit_ys(j, pT, off, g0, psYt=psYt, psSt=psSt, c=c,
                                nj=nj):
                        nc.tensor.matmul(
                            psYt[:, off:512],
                            vnat[:, b * (S // 128) + j, 128 * hl:128 * (hl + 1)],
                            pT[:, off:512],
                            start=(j == 0), stop=(j == nj - 1),
                            skip_group_check=True)
                        for g in range(g0, 4):
                            # start only on the very first sums matmul of the
                            # chunk (bank-wide zero region); later columns
                            # accumulate onto pending-zero bytes.
                            nc.tensor.matmul(
                                psSt[:, g:g + 1],
                                pT[:, 128 * g:128 * (g + 1)], ones_col,
                                start=(j == 0 and g == 0),
                                stop=(j == nj - 1 and g == 3),
                                skip_group_check=True)

                    for j in range(nj):
                        g0 = max(0, j - 4 * c)
                        off = 128 * g0
                        ps = psA.tile([128, 512], F32, tag="A")
                        nc.tensor.matmul(
                            ps[:, off:512],
                            k_t[:, 128 * j:128 * (j + 1)],
                            q_t[:, c * 512 + off:(c + 1) * 512],
                            start=True, stop=True, skip_group_check=True)
                        if j >= 4 * c:
                            nc.vector.tensor_add(
                                ps[:, off:off + 128], ps[:, off:off + 128],
                                mask_sb)
                        pT = ptp.tile([128, 512], BF16, tag="p")
                        nc.scalar.activation(
                            pT[:, off:512], ps[:, off:512], AF.Exp,
                            scale=cfg.SCALE)
                        if prev is not None:
                            emit_ys(*prev)
                        if j == 0 and pending[0] is not None:
                            pending[0]()
                            pending[0] = None
                        feeder.step()
                        prev = (j, pT, off, g0)
                    emit_ys(*prev)
                    pending[0] = (lambda c=c, y=psYt, s=psSt:
                                  emit_chain(c, y, s))
                if pending[0] is not None:
                    pending[0]()
                    pending[0] = None

            for b in range(B):
                for hl in range(HPC):
                    attention(b, hl)
                    if hl == HPC - 1:
                        for scp in range(cfg.NQC // 2):
                            feeder.push(dense_steps(b, scp))
            feeder.drain()

            if debug:
                dqk = nc.dram_tensor("dbg_qk", [128, NQK, SF], BF16,
                                     kind="ExternalOutput")
                dv = nc.dram_tensor("dbg_v", [128, NT, VW], BF16,
                                    kind="ExternalOutput")
                dy = nc.dram_tensor("dbg_y", [128, CT, SF], BF16,
                                    kind="ExternalOutput")
                nc.sync.dma_start(out=dqk[:, :, :], in_=qk_sb[:, :, :])
                nc.sync.dma_start(out=dv[:, :, :], in_=vnat[:, :, :])
                nc.sync.dma_start(out=dy[:, :, :], in_=yT_sb[:, :, :])

    nc.finalize()
    return nc


# ---------------------------------------------------------------------------
# Host-side input preparation / sharding
# ---------------------------------------------------------------------------

def _bf16(a):
    import ml_dtypes
    return np.ascontiguousarray(a, np.float32).astype(ml_dtypes.bfloat16)


def _rope_tables(cfg: Cfg):
    inv_freq = 1.0 / (10000.0 ** (np.arange(0, cfg.ROT, 2, dtype=np.float64)
                                  / cfg.ROT))
    t = np.arange(cfg.S, dtype=np.float64)
    freqs = np.outer(t, inv_freq)                       # [S, 16]
    emb = np.concatenate([freqs, freqs], axis=-1)       # [S, 32]
    cos = np.cos(emb).T.astype(np.float32)              # [32, S]
    sin = np.sin(emb).T.astype(np.float32)
    cosF = np.tile(cos, (1, cfg.B))                     # [32, SF]
    sinF = np.tile(sin, (1, cfg.B))
    sinF[:cfg.ROT // 2] *= -1.0                         # fold rotate_half sign
    G = cfg.G
    cos128 = np.ascontiguousarray(
        cosF.reshape(32, 4, G).transpose(1, 0, 2).reshape(128, G))
    sin128s = np.ascontiguousarray(
        sinF.reshape(32, 4, G).transpose(1, 0, 2).reshape(128, G))
    return _bf16(cos128), _bf16(sin128s)


def make_in_maps(cfg: Cfg, x, w_qkv, b_qkv, w_dense):
    HS, HPC = cfg.HS, cfg.HPC
    xTb = _bf16(np.ascontiguousarray(
        np.asarray(x, np.float32).reshape(cfg.SF, cfg.E).T))
    cos128, sin128s = _rope_tables(cfg)
    p = np.arange(128)[:, None]
    f = np.arange(128)[None, :]
    maskT = np.where(p <= f, 0.0, NEG_MASK).astype(np.float32)
    in_maps = []
    for i in range(cfg.n_cores):
        heads = [HPC * i + h for h in range(HPC)]
        qk_rows = np.concatenate(
            [np.arange(h * 3 * HS + qk * HS, h * 3 * HS + (qk + 1) * HS)
             for h in heads for qk in range(2)])
        v_rows = np.concatenate(
            [np.arange(h * 3 * HS + 2 * HS, h * 3 * HS + 3 * HS)
             for h in heads])
        wcat = np.concatenate(
            [np.asarray(w_qkv, np.float32)[qk_rows, :].T,
             np.asarray(w_qkv, np.float32)[v_rows, :].T], axis=1)
        cols = slice(i * cfg.VW, (i + 1) * cfg.VW)
        in_maps.append({
            "xT": xTb,
            "wcat": _bf16(wcat),
            "bqk": np.ascontiguousarray(
                np.asarray(b_qkv, np.float32)[qk_rows]),
            "vbbc": np.ascontiguousarray(np.tile(
                np.asarray(b_qkv, np.float32)[v_rows][None, :], (128, 1))),
            "wdT": _bf16(np.asarray(w_dense, np.float32)[:, cols].T),
            "cosT": cos128,
            "sinT": sin128s,
            "maskT": maskT,
        })
    return in_maps


def combine_outputs(cfg: Cfg, results, b_dense):
    acc = np.zeros((cfg.E, cfg.SF), dtype=np.float64)
    for r in results:
        acc += np.asarray(r["outT"]).astype(np.float64)
    out = acc.T.reshape(cfg.B, cfg.S, cfg.E) + \
        np.asarray(b_dense, np.float64)
    return out.astype(np.float32)


_PROGRAM_CACHE = {}


def kernel(x, w_qkv, b_qkv, w_dense, b_dense):
    from concourse.bass_utils import run_bass_kernel_spmd

    cfg = Cfg()
    key = "full"
    if key not in _PROGRAM_CACHE:
        _PROGRAM_CACHE[key] = build_program(cfg)
    nc = _PROGRAM_CACHE[key]
    in_maps = make_in_maps(cfg, np.asarray(x), np.asarray(w_qkv),
                           np.asarray(b_qkv), np.asarray(w_dense))
    res = run_bass_kernel_spmd(nc, in_maps, list(range(cfg.n_cores)))
    return combine_outputs(cfg, res.results, np.asarray(b_dense))


# revision 14
# speedup vs baseline: 1.3525x; 1.0052x over previous
"""GPT-NeoX attention layer (B=2, S=2048, E=2048, H=16, partial RoPE 32/128)
as a Bass/Tile kernel for 8 Trainium2 NeuronCores.

Sharding: tensor-parallel across heads (2 heads per core, Megatron-style).
Each core computes QKV projection for its 2 heads over all tokens, applies
partial RoPE, runs causal attention, and produces a partial dense output
(contraction over its 256 columns of w_dense).  The 8 bf16 partial outputs
are summed on the host and the dense bias is added once on the host.

Everything on-device is bf16 (inputs pre-converted on the host); PSUM
accumulation stays fp32.  Key structure choices:

  qk_sb  [128, 4, SF]   Q^T/K^T per head (head dim on partitions) - scores
                        and y^T matmuls consume this directly.
  vnat   [128, SF/128, 256]  V in NATURAL [token, d] layout, produced in
                        phase 1 by x-stationary matmuls (x as lhsT), so no
                        PE transposes of V are ever needed.
  scores S^T = K^T.T @ Q^T in [sk, sq] blocks; exp on ScalarE (pipelined one
                        block behind the scores matmuls).
  softmax sums          via N=1 matmuls with the exp'd block as the
                        stationary operand (out [sq,1]): nearly free on PE,
                        instead of a 512-wide ones-matmul per block.
  normalize             reciprocal -> tiny PE transpose -> GPSIMD
                        partition_broadcast -> one DVE multiply per chunk.
  dense                 interleaved into later attention heads (fills the
                        tensor engine while ScalarE works through exp).
"""

import numpy as np
from contextlib import ExitStack

import concourse.bass as bass
import concourse.bacc as bacc
import concourse.mybir as mybir
import concourse.tile as tile
from concourse.masks import make_identity

AF = mybir.ActivationFunctionType
F32 = mybir.dt.float32
BF16 = mybir.dt.bfloat16

NEG_MASK = -1.0e9


class Cfg:
    def __init__(self, B=2, S=2048, E=2048, H=16, n_cores=8):
        self.B, self.S, self.E, self.H = B, S, E, H
        self.HS = 128                  # head size (one partition tile)
        self.ROT = 32                  # rotary dims
        self.n_cores = n_cores
        self.HPC = H // n_cores        # heads per core
        assert self.HPC == 2, "kernel assumes 2 heads per core"
        self.NQK = 2 * self.HPC        # q/k row tiles (h0q,h0k,h1q,h1k)
        self.VW = self.HPC * self.HS   # v natural width (d per core)
        self.RW = self.NQK * self.HS   # per-core q+k rows
        self.WCOLS = self.RW + self.VW
        self.SF = B * S
        self.KT = E // 128             # contraction tiles
        self.SC = 256                  # phase-1 token chunk
        self.NP1 = self.SF // self.SC
        self.G = self.SF // 4          # rope regroup width
        self.NQC = S // 512            # q chunks per (b, h)
        self.EO = E // 128             # dense output row tiles
        self.CT = self.HPC             # dense contraction tiles
        self.SCALE = 1.0 / np.sqrt(self.HS)
        assert S % 512 == 0 and E % 128 == 0 and self.SF % (4 * self.SC) == 0


class _Feeder:
    """Round-robin sink of deferred emission micro-steps (dense tiles)."""

    def __init__(self):
        self.gens = []

    def push(self, gen):
        self.gens.append(gen)

    def step(self):
        while self.gens:
            try:
                next(self.gens[0])
                return
            except StopIteration:
                self.gens.pop(0)

    def drain(self):
        while self.gens:
            g = self.gens.pop(0)
            for _ in g:
                pass


def build_program(cfg: Cfg, debug: bool = False) -> bass.Bass:
    B, S, E = cfg.B, cfg.S, cfg.E
    SF, KT, G = cfg.SF, cfg.KT, cfg.G
    SC, NQK, VW, RW = cfg.SC, cfg.NQK, cfg.VW, cfg.RW
    HPC, CT, EO = cfg.HPC, cfg.CT, cfg.EO
    NT = SF // 128                   # vnat token tiles

    nc = bacc.Bacc(None)
    xT = nc.dram_tensor("xT", [E, SF], BF16, kind="ExternalInput")
    wcat = nc.dram_tensor("wcat", [E, cfg.WCOLS], BF16, kind="ExternalInput")
    bqk = nc.dram_tensor("bqk", [RW], F32, kind="ExternalInput")
    vbbc = nc.dram_tensor("vbbc", [128, VW], F32, kind="ExternalInput")
    wdT = nc.dram_tensor("wdT", [VW, E], BF16, kind="ExternalInput")
    cosT = nc.dram_tensor("cosT", [128, G], BF16, kind="ExternalInput")
    sinT = nc.dram_tensor("sinT", [128, G], BF16, kind="ExternalInput")
    maskT = nc.dram_tensor("maskT", [128, 128], F32, kind="ExternalInput")
    outT = nc.dram_tensor("outT", [E, SF], BF16, kind="ExternalOutput")

    with tile.TileContext(nc) as tc, ExitStack() as stk:
        consts = stk.enter_context(tc.tile_pool(name="consts", bufs=1))
        bigp = stk.enter_context(tc.tile_pool(name="big", bufs=1))
        qk_sb = bigp.tile([128, NQK, SF], BF16)
        vnat = bigp.tile([128, NT, VW], BF16)
        yT_sb = bigp.tile([128, CT, SF], BF16)

        # ---------------- Phase 1: QKV projection ------------------------
        with tc.tile_pool(name="wq", bufs=1) as wp, \
             tc.tile_pool(name="xs", bufs=2) as xp, \
             tc.tile_pool(name="ps1", bufs=2, space="PSUM") as pp1:
            w_sb = wp.tile([128, KT, cfg.WCOLS], BF16)
            w_view = wcat.rearrange("(kt p) r -> p kt r", p=128)
            x_view = xT.rearrange("(kt p) s -> p kt s", p=128)

            # interleave per-kt w loads with quarters of the first x chunk
            xt0 = xp.tile([128, KT, SC], BF16, tag="xt")
            ktg = max(1, KT // 4)
            for q0 in range(0, KT, ktg):
                q1 = min(q0 + ktg, KT)
                for kt in range(q0, q1):
                    nc.sync.dma_start(out=w_sb[:, kt, :], in_=w_view[:, kt, :])
                nc.sync.dma_start(out=xt0[:, q0:q1, :],
                                  in_=x_view[:, q0:q1, 0:SC])
            xt1 = xp.tile([128, KT, SC], BF16, tag="xt")
            nc.sync.dma_start(out=xt1[:, :, :], in_=x_view[:, :, SC:2 * SC])

            # constants (after the critical w/x stream)
            ident = consts.tile([128, 128], F32)
            make_identity(nc, ident)
            identB = consts.tile([128, 128], BF16)
            with nc.allow_low_precision(reason="bf16 identity"):
                nc.vector.tensor_copy(identB, ident)
            ones_tmp = consts.tile([128, 1], BF16, tag="onestmp")
            nc.vector.memset(ones_tmp, 1.0)
            ones_col = ones_tmp
            mask_sb = consts.tile([128, 128], F32)
            nc.sync.dma_start(out=mask_sb, in_=maskT[:, :])
            bqk_sb = consts.tile([128, NQK], F32)
            nc.sync.dma_start(out=bqk_sb,
                              in_=bqk.rearrange("(rt p) -> p rt", p=128))
            vb_sb = consts.tile([128, VW], F32)
            nc.sync.dma_start(out=vb_sb, in_=vbbc[:, :])
            cos_sb = consts.tile([128, G], BF16, tag="costab")
            sin_sb = consts.tile([128, G], BF16, tag="sintab")
            nc.sync.dma_start(out=cos_sb, in_=cosT[:, :])
            nc.sync.dma_start(out=sin_sb, in_=sinT[:, :])
            wd_sb = consts.tile([128, CT, E], BF16, tag="wd")
            nc.sync.dma_start(
                out=wd_sb[:, :, :],
                in_=wdT.rearrange("(ct p) e -> p ct e", p=128))

            ntile = SC // 128   # v token sub-tiles per chunk (=2)
            for sc in range(cfg.NP1):
                if sc == 0:
                    xt = xt0
                elif sc == 1:
                    xt = xt1
                else:
                    xt = xp.tile([128, KT, SC], BF16, tag="xt")
                    nc.sync.dma_start(
                        out=xt[:, :, :],
                        in_=x_view[:, :, sc * SC:(sc + 1) * SC])
                qkps = [pp1.tile([128, 512], F32, tag=f"qk{i}", name=f"qkps{i}")
                        for i in range(NQK // 2)]
                vps = pp1.tile([128, 512], F32, tag="v")
                # NOTE: a start=True matmul marks the PSUM bank's whole 2KB
                # zero-region pending-zero, so only the FIRST matmul into
                # each bank starts; co-resident column groups accumulate
                # onto pending-zero bytes.
                for kt in range(KT):
                    fl, ll = (kt == 0), (kt == KT - 1)
                    for i in range(NQK):
                        nc.tensor.matmul(
                            qkps[i // 2][:, 256 * (i % 2):256 * (i % 2) + SC],
                            w_sb[:, kt, 128 * i:128 * (i + 1)],
                            xt[:, kt, :],
                            start=fl and i % 2 == 0,
                            stop=ll and i % 2 == 1, skip_group_check=True)
                    for t in range(ntile):
                        nc.tensor.matmul(
                            vps[:, VW * t:VW * (t + 1)],
                            xt[:, kt, 128 * t:128 * (t + 1)],
                            w_sb[:, kt, RW:RW + VW],
                            start=fl and t == 0,
                            stop=ll and t == ntile - 1, skip_group_check=True)
                for i in range(NQK):
                    nc.scalar.activation(
                        qk_sb[:, i, sc * SC:(sc + 1) * SC],
                        qkps[i // 2][:, 256 * (i % 2):256 * (i % 2) + SC],
                        AF.Identity, bias=bqk_sb[:, i:i + 1])
                with nc.allow_low_precision(reason="bf16 v eviction"):
                    for t in range(ntile):
                        nc.vector.tensor_add(
                            vnat[:, sc * ntile + t, :],
                            vps[:, VW * t:VW * (t + 1)], vb_sb)

        # ---------------- RoPE on first 32 rows of each Q^T/K^T ----------
        # Per-head pattern: issue ALL regroup loads for the head's q and k
        # tiles before any combine, so the (SEQ-blocking) writeback waits
        # don't stall the next tile's loads.  Head 0's combines run on DVE
        # (fast: the first attention head waits on them anyway); later
        # heads' combines run on idle GPSIMD so they cannot delay the
        # attention mask adds queued behind them on DVE.
        half = cfg.ROT // 2
        with tc.tile_pool(name="rope", bufs=2) as rp:
            for h in range(HPC):
                eng = nc.vector if h == 0 else nc.gpsimd
                tiles = []
                for qk in range(2):
                    i = 2 * h + qk
                    plain = rp.tile([128, G], BF16, tag=f"plain{qk}",
                                    name=f"plain{qk}")
                    sw = rp.tile([128, G], BF16, tag=f"swap{qk}",
                                 name=f"sw{qk}")
                    for g in range(4):
                        gs = slice(g * G, (g + 1) * G)
                        nc.sync.dma_start(
                            out=plain[g * 32:(g + 1) * 32, :],
                            in_=qk_sb[0:cfg.ROT, i, gs])
                        nc.sync.dma_start(
                            out=sw[g * 32:g * 32 + half, :],
                            in_=qk_sb[half:cfg.ROT, i, gs])
                        nc.sync.dma_start(
                            out=sw[g * 32 + half:(g + 1) * 32, :],
                            in_=qk_sb[0:half, i, gs])
                    tiles.append((i, plain, sw))
                for i, plain, sw in tiles:
                    with nc.allow_low_precision(reason="bf16 rope"):
                        eng.tensor_mul(plain, plain, cos_sb)
                        eng.tensor_mul(sw, sw, sin_sb)  # sign folded in
                        eng.tensor_add(plain, plain, sw)
                    for g in range(4):
                        gs = slice(g * G, (g + 1) * G)
                        nc.sync.dma_start(
                            out=qk_sb[0:cfg.ROT, i, gs],
                            in_=plain[g * 32:(g + 1) * 32, :])

        # ---------------- Attention + interleaved dense -------------------
        feeder = _Feeder()
        with tc.tile_pool(name="pstrip", bufs=3) as ptp, \
             tc.tile_pool(name="norm", bufs=2) as npool, \
             tc.tile_pool(name="outsb", bufs=4) as op, \
             tc.tile_pool(name="psA", bufs=3, space="PSUM") as psA, \
             tc.tile_pool(name="psY", bufs=2, space="PSUM") as psY, \
             tc.tile_pool(name="psS", bufs=1, space="PSUM") as psS, \
             tc.tile_pool(name="psD", bufs=2, space="PSUM") as psD:

            def dense_steps(b, scp):
                # one (eo) output row-tile over two 512-token col chunks
                for eo in range(EO):
                    ot = op.tile([128, 1024], BF16, tag="out")
                    for t in range(2):
                        col = b * S + (2 * scp + t) * 512
                        pd = psD.tile([128, 512], F32, tag="D")
                        for ct in range(CT):
                            nc.tensor.matmul(
                                pd,
                                wd_sb[:, ct, 128 * eo:128 * (eo + 1)],
                                yT_sb[:, ct, col:col + 512],
                                start=(ct == 0), stop=(ct == CT - 1),
                                skip_group_check=True)
                        yield
                        with nc.allow_low_precision(reason="bf16 out"):
                            if (eo + t) % 2 == 0:
                                nc.vector.tensor_copy(
                                    ot[:, 512 * t:512 * (t + 1)], pd)
                            else:
                                nc.scalar.activation(
                                    ot[:, 512 * t:512 * (t + 1)], pd, AF.Copy)
                    nc.sync.dma_start(
                        out=outT[128 * eo:128 * (eo + 1),
                                 b * S + scp * 1024:b * S + (scp + 1) * 1024],
                        in_=ot)
                    yield

            def attention(b, hl):
                scol = b * S
                q_t = qk_sb[:, 2 * hl, scol:scol + S]
                k_t = qk_sb[:, 2 * hl + 1, scol:scol + S]
                pending = [None]   # chunk-end normalization closure

                def emit_chain(c, psYt, psSt):
                    recip = npool.tile([128, 4], F32, tag="recip")
                    nc.vector.reciprocal(recip, psSt[:, 0:4])
                    # transpose each recip column to partition 0 ([1, 128])
                    rps = psA.tile([128, 512], F32, tag="A", name="rps")
                    for g in range(4):
                        nc.tensor.matmul(
                            rps[0:1, 128 * g:128 * (g + 1)],
                            recip[:, g:g + 1], ident,
                            is_transpose=True, start=(g == 0), stop=(g == 3),
                            skip_group_check=True)
                    rT = npool.tile([1, 512], F32, tag="rT")
                    nc.vector.tensor_copy(rT, rps[0:1, 0:512])
                    bc = npool.tile([128, 512], F32, tag="bc")
                    for g in range(4):
                        nc.gpsimd.partition_broadcast(
                            bc[:, 128 * g:128 * (g + 1)],
                            rT[0:1, 128 * g:128 * (g + 1)])
                    with nc.allow_low_precision(reason="bf16 y eviction"):
                        nc.vector.tensor_mul(
                            yT_sb[:, hl, scol + c * 512:scol + (c + 1) * 512],
                            psYt[:, 0:512], bc)

                for c in range(cfg.NQC):
                    nj = 4 * (c + 1)
                    psYt = psY.tile([128, 512], F32, tag="Y")
                    psSt = psS.tile([128, 4], F32, tag="S")
                    prev = None

                    def emit_ys(j, pT, off, g0, psYt=psYt, psSt=psSt, c=c,
                                nj=nj):
                        nc.tensor.matmul(
                            psYt[:, off:512],
                            vnat[:, b * (S // 128) + j, 128 * hl:128 * (hl + 1)],
                            pT[:, off:512],
                            start=(j == 0), stop=(j == nj - 1),
                            skip_group_check=True)
                        for g in range(g0, 4):
                            # start only on the very first sums matmul of the
                            # chunk (bank-wide zero region); later columns
                            # accumulate onto pending-zero bytes.
                            nc.tensor.matmul(
                                psSt[:, g:g + 1],
                                pT[:, 128 * g:128 * (g + 1)], ones_col,
                                start=(j == 0 and g == 0),
                                stop=(j == nj - 1 and g == 3),
                                skip_group_check=True)

                    for j in range(nj):
                        g0 = max(0, j - 4 * c)
                        off = 128 * g0
                        ps = psA.tile([128, 512], F32, tag="A")
                        nc.tensor.matmul(
                            ps[:, off:512],
                            k_t[:, 128 * j:128 * (j + 1)],
                            q_t[:, c * 512 + off:(c + 1) * 512],
                            start=True, stop=True, skip_group_check=True)
                        if j >= 4 * c:
                            nc.vector.tensor_add(
                                ps[:, off:off + 128], ps[:, off:off + 128],
                                mask_sb)
                        pT = ptp.tile([128, 512], BF16, tag="p")
                        nc.scalar.activation(
                            pT[:, off:512], ps[:, off:512], AF.Exp,
                            scale=cfg.SCALE)
                        if prev is not None:
                            emit_ys(*prev)
                        if j == 0 and pending[0] is not None:
                            pending[0]()
                            pending[0] = None
                        feeder.step()
                        prev = (j, pT, off, g0)
                    emit_ys(*prev)
                    pending[0] = (lambda c=c, y=psYt, s=psSt:
                                  emit_chain(c, y, s))
                if pending[0] is not None:
                    pending[0]()
                    pending[0] = None

            for b in range(B):
                for hl in range(HPC):
                    attention(b, hl)
                    if hl == HPC - 1:
                        for scp in range(cfg.NQC // 2):
                            feeder.push(dense_steps(b, scp))
            feeder.drain()

            if debug:
                dqk = nc.dram_tensor("dbg_qk", [128, NQK, SF], BF16,
                                     kind="ExternalOutput")
                dv = nc.dram_tensor("dbg_v", [128, NT, VW], BF16,
                                    kind="ExternalOutput")
                dy = nc.dram_tensor("dbg_y", [128, CT, SF], BF16,
                                    kind="ExternalOutput")
                nc.sync.dma_start(out=dqk[:, :, :], in_=qk_sb[:, :, :])
                nc.sync.dma_start(out=dv[:, :, :], in_=vnat[:, :, :])
                nc.sync.dma_start(out=dy[:, :, :], in_=yT_sb[:, :, :])

    nc.finalize()
    return nc


# ---------------------------------------------------------------------------
# Host-side input preparation / sharding
# ---------------------------------------------------------------------------

def _bf16(a):
    import ml_dtypes
    return np.ascontiguousarray(a, np.float32).astype(ml_dtypes.bfloat16)


def _rope_tables(cfg: Cfg):
    inv_freq = 1.0 / (10000.0 ** (np.arange(0, cfg.ROT, 2, dtype=np.float64)
                                  / cfg.ROT))
    t = np.arange(cfg.S, dtype=np.float64)
    freqs = np.outer(t, inv_freq)                       # [S, 16]
    emb = np.concatenate([freqs, freqs], axis=-1)       # [S, 32]
    cos = np.cos(emb).T.astype(np.float32)              # [32, S]
    sin = np.sin(emb).T.astype(np.float32)
    cosF = np.tile(cos, (1, cfg.B))                     # [32, SF]
    sinF = np.tile(sin, (1, cfg.B))
    sinF[:cfg.ROT // 2] *= -1.0                         # fold rotate_half sign
    G = cfg.G
    cos128 = np.ascontiguousarray(
        cosF.reshape(32, 4, G).transpose(1, 0, 2).reshape(128, G))
    sin128s = np.ascontiguousarray(
        sinF.reshape(32, 4, G).transpose(1, 0, 2).reshape(128, G))
    return _bf16(cos128), _bf16(sin128s)


def make_in_maps(cfg: Cfg, x, w_qkv, b_qkv, w_dense):
    HS, HPC = cfg.HS, cfg.HPC
    xTb = _bf16(np.ascontiguousarray(
        np.asarray(x, np.float32).reshape(cfg.SF, cfg.E).T))
    cos128, sin128s = _rope_tables(cfg)
    p = np.arange(128)[:, None]
    f = np.arange(128)[None, :]
    maskT = np.where(p <= f, 0.0, NEG_MASK).astype(np.float32)
    in_maps = []
    for i in range(cfg.n_cores):
        heads = [HPC * i + h for h in range(HPC)]
        qk_rows = np.concatenate(
            [np.arange(h * 3 * HS + qk * HS, h * 3 * HS + (qk + 1) * HS)
             for h in heads for qk in range(2)])
        v_rows = np.concatenate(
            [np.arange(h * 3 * HS + 2 * HS, h * 3 * HS + 3 * HS)
             for h in heads])
        wcat = np.concatenate(
            [np.asarray(w_qkv, np.float32)[qk_rows, :].T,
             np.asarray(w_qkv, np.float32)[v_rows, :].T], axis=1)
        cols = slice(i * cfg.VW, (i + 1) * cfg.VW)
        in_maps.append({
            "xT": xTb,
            "wcat": _bf16(wcat),
            "bqk": np.ascontiguousarray(
                np.asarray(b_qkv, np.float32)[qk_rows]),
            "vbbc": np.ascontiguousarray(np.tile(
                np.asarray(b_qkv, np.float32)[v_rows][None, :], (128, 1))),
            "wdT": _bf16(np.asarray(w_dense, np.float32)[:, cols].T),
            "cosT": cos128,
            "sinT": sin128s,
            "maskT": maskT,
        })
    return in_maps


def combine_outputs(cfg: Cfg, results, b_dense):
    acc = np.zeros((cfg.E, cfg.SF), dtype=np.float64)
    for r in results:
        acc += np.asarray(r["outT"]).astype(np.float64)
    out = acc.T.reshape(cfg.B, cfg.S, cfg.E) + \
        np.asarray(b_dense, np.float64)
    return out.astype(np.float32)


_PROGRAM_CACHE = {}


def kernel(x, w_qkv, b_qkv, w_dense, b_dense):
    from concourse.bass_utils import run_bass_kernel_spmd

    cfg = Cfg()
    key = "full"
    if key not in _PROGRAM_CACHE:
        _PROGRAM_CACHE[key] = build_program(cfg)
    nc = _PROGRAM_CACHE[key]
    in_maps = make_in_maps(cfg, np.asarray(x), np.asarray(w_qkv),
                           np.asarray(b_qkv), np.asarray(w_dense))
    res = run_bass_kernel_spmd(nc, in_maps, list(range(cfg.n_cores)))
    return combine_outputs(cfg, res.results, np.asarray(b_dense))


# revision 20
# speedup vs baseline: 1.4656x; 1.0836x over previous
"""GPT-NeoX attention layer (B=2, S=2048, E=2048, H=16, partial RoPE 32/128)
as a Bass/Tile kernel for 8 Trainium2 NeuronCores.

Sharding: tensor-parallel across heads (2 heads per core, Megatron-style).
Each core computes QKV projection for its 2 heads over all tokens, applies
partial RoPE, runs causal attention, and produces a partial dense output
(contraction over its 256 columns of w_dense).  The 8 bf16 partial outputs
are summed on the host and the dense bias is added once on the host.

Everything on-device is bf16 (inputs pre-converted on the host); PSUM
accumulation stays fp32.  Key structure choices:

  qk_sb  [128, 4, SF]   Q^T/K^T per head (head dim on partitions) - scores
                        and y^T matmuls consume this directly.
  vnat   [128, SF/128, 256]  V in NATURAL [token, d] layout, produced in
                        phase 1 by x-stationary matmuls (x as lhsT), so no
                        PE transposes of V are ever needed.
  scores S^T = K^T.T @ Q^T in [sk, sq] blocks; exp on ScalarE (pipelined one
                        block behind the scores matmuls).
  softmax sums          via N=1 matmuls with the exp'd block as the
                        stationary operand (out [sq,1]): nearly free on PE,
                        instead of a 512-wide ones-matmul per block.
  normalize             reciprocal -> tiny PE transpose -> GPSIMD
                        partition_broadcast -> one DVE multiply per chunk.
  dense                 interleaved into later attention heads (fills the
                        tensor engine while ScalarE works through exp).
"""

import numpy as np
from contextlib import ExitStack

import concourse.bass as bass
import concourse.bacc as bacc
import concourse.mybir as mybir
import concourse.tile as tile
from concourse.masks import make_identity

AF = mybir.ActivationFunctionType
F32 = mybir.dt.float32
BF16 = mybir.dt.bfloat16

NEG_MASK = -1.0e9


class Cfg:
    def __init__(self, B=2, S=2048, E=2048, H=16, n_cores=8):
        self.B, self.S, self.E, self.H = B, S, E, H
        self.HS = 128                  # head size (one partition tile)
        self.ROT = 32                  # rotary dims
        self.n_cores = n_cores
        self.HPC = H // n_cores        # heads per core
        assert self.HPC == 2, "kernel assumes 2 heads per core"
        self.NQK = 2 * self.HPC        # q/k row tiles (h0q,h0k,h1q,h1k)
        self.VW = self.HPC * self.HS   # v natural width (d per core)
        self.RW = self.NQK * self.HS   # per-core q+k rows
        self.WCOLS = self.RW + self.VW
        self.SF = B * S
        self.KT = E // 128             # contraction tiles
        self.SC = 256                  # phase-1 token chunk
        self.NP1 = self.SF // self.SC
        self.G = self.SF // 4          # rope regroup width
        self.NQC = S // 512            # q chunks per (b, h)
        self.EO = E // 128             # dense output row tiles
        self.CT = self.HPC             # dense contraction tiles
        self.SCALE = 1.0 / np.sqrt(self.HS)
        assert S % 512 == 0 and E % 128 == 0 and self.SF % (4 * self.SC) == 0


class _Feeder:
    """Round-robin sink of deferred emission micro-steps (dense tiles)."""

    def __init__(self):
        self.gens = []

    def push(self, gen):
        self.gens.append(gen)

    def step(self):
        while self.gens:
            try:
                next(self.gens[0])
                return
            except StopIteration:
                self.gens.pop(0)

    def drain(self):
        while self.gens:
            g = self.gens.pop(0)
            for _ in g:
                pass


def build_program(cfg: Cfg, debug: bool = False) -> bass.Bass:
    B, S, E = cfg.B, cfg.S, cfg.E
    SF, KT, G = cfg.SF, cfg.KT, cfg.G
    SC, NQK, VW, RW = cfg.SC, cfg.NQK, cfg.VW, cfg.RW
    HPC, CT, EO = cfg.HPC, cfg.CT, cfg.EO
    NT = SF // 128                   # vnat token tiles

    nc = bacc.Bacc(None)
    xT = nc.dram_tensor("xT", [E, SF], BF16, kind="ExternalInput")
    wcat = nc.dram_tensor("wcat", [E, cfg.WCOLS], BF16, kind="ExternalInput")
    bqk = nc.dram_tensor("bqk", [RW], F32, kind="ExternalInput")
    vbbc = nc.dram_tensor("vbbc", [128, VW], F32, kind="ExternalInput")
    wdT = nc.dram_tensor("wdT", [VW, E], BF16, kind="ExternalInput")
    cosT = nc.dram_tensor("cosT", [32, SF], BF16, kind="ExternalInput")
    sinT = nc.dram_tensor("sinT", [32, SF], BF16, kind="ExternalInput")
    maskT = nc.dram_tensor("maskT", [128, 128], F32, kind="ExternalInput")
    outT = nc.dram_tensor("outT", [E, SF], BF16, kind="ExternalOutput")

    with tile.TileContext(nc) as tc, ExitStack() as stk:
        consts = stk.enter_context(tc.tile_pool(name="consts", bufs=1))
        bigp = stk.enter_context(tc.tile_pool(name="big", bufs=1))
        qk_sb = bigp.tile([128, NQK, SF], BF16)
        vnat = bigp.tile([128, NT, VW], BF16)
        yT_sb = bigp.tile([128, CT, SF], BF16)

        # ---------------- Phase 1: QKV projection ------------------------
        with tc.tile_pool(name="wq", bufs=1) as wp, \
             tc.tile_pool(name="xs", bufs=2) as xp, \
             tc.tile_pool(name="ps1", bufs=2, space="PSUM") as pp1:
            w_sb = wp.tile([128, KT, cfg.WCOLS], BF16)
            w_view = wcat.rearrange("(kt p) r -> p kt r", p=128)
            x_view = xT.rearrange("(kt p) s -> p kt s", p=128)

            # interleave per-kt w loads with quarters of the first x chunk
            xt0 = xp.tile([128, KT, SC], BF16, tag="xt")
            ktg = max(1, KT // 4)
            for q0 in range(0, KT, ktg):
                q1 = min(q0 + ktg, KT)
                for kt in range(q0, q1):
                    nc.sync.dma_start(out=w_sb[:, kt, :], in_=w_view[:, kt, :])
                nc.sync.dma_start(out=xt0[:, q0:q1, :],
                                  in_=x_view[:, q0:q1, 0:SC])
            xt1 = xp.tile([128, KT, SC], BF16, tag="xt")
            nc.sync.dma_start(out=xt1[:, :, :], in_=x_view[:, :, SC:2 * SC])

            # constants (after the critical w/x stream)
            ident = consts.tile([128, 128], F32)
            make_identity(nc, ident)
            identB = consts.tile([128, 128], BF16)
            with nc.allow_low_precision(reason="bf16 identity"):
                nc.vector.tensor_copy(identB, ident)
            ones_tmp = consts.tile([128, 1], BF16, tag="onestmp")
            nc.vector.memset(ones_tmp, 1.0)
            ones_col = ones_tmp
            mask_sb = consts.tile([128, 128], F32)
            nc.sync.dma_start(out=mask_sb, in_=maskT[:, :])
            bqk_sb = consts.tile([128, NQK], F32)
            nc.sync.dma_start(out=bqk_sb,
                              in_=bqk.rearrange("(rt p) -> p rt", p=128))
            vb_sb = consts.tile([128, VW], F32)
            nc.sync.dma_start(out=vb_sb, in_=vbbc[:, :])
            cos_sb = consts.tile([32, SF], BF16, tag="costab")
            sin_sb = consts.tile([32, SF], BF16, tag="sintab")
            nc.sync.dma_start(out=cos_sb, in_=cosT[:, :])
            nc.sync.dma_start(out=sin_sb, in_=sinT[:, :])
            wd_sb = consts.tile([128, CT, E], BF16, tag="wd")
            nc.sync.dma_start(
                out=wd_sb[:, :, :],
                in_=wdT.rearrange("(ct p) e -> p ct e", p=128))

            ntile = SC // 128   # v token sub-tiles per chunk (=2)
            for sc in range(cfg.NP1):
                if sc == 0:
                    xt = xt0
                elif sc == 1:
                    xt = xt1
                else:
                    xt = xp.tile([128, KT, SC], BF16, tag="xt")
                    nc.sync.dma_start(
                        out=xt[:, :, :],
                        in_=x_view[:, :, sc * SC:(sc + 1) * SC])
                qkps = [pp1.tile([128, 512], F32, tag=f"qk{i}", name=f"qkps{i}")
                        for i in range(NQK // 2)]
                vps = pp1.tile([128, 512], F32, tag="v")
                # NOTE: a start=True matmul marks the PSUM bank's whole 2KB
                # zero-region pending-zero, so only the FIRST matmul into
                # each bank starts; co-resident column groups accumulate
                # onto pending-zero bytes.
                for kt in range(KT):
                    fl, ll = (kt == 0), (kt == KT - 1)
                    for i in range(NQK):
                        nc.tensor.matmul(
                            qkps[i // 2][:, 256 * (i % 2):256 * (i % 2) + SC],
                            w_sb[:, kt, 128 * i:128 * (i + 1)],
                            xt[:, kt, :],
                            start=fl and i % 2 == 0,
                            stop=ll and i % 2 == 1, skip_group_check=True)
                    for t in range(ntile):
                        nc.tensor.matmul(
                            vps[:, VW * t:VW * (t + 1)],
                            xt[:, kt, 128 * t:128 * (t + 1)],
                            w_sb[:, kt, RW:RW + VW],
                            start=fl and t == 0,
                            stop=ll and t == ntile - 1, skip_group_check=True)
                for i in range(NQK):
                    nc.scalar.activation(
                        qk_sb[:, i, sc * SC:(sc + 1) * SC],
                        qkps[i // 2][:, 256 * (i % 2):256 * (i % 2) + SC],
                        AF.Identity, bias=bqk_sb[:, i:i + 1])
                with nc.allow_low_precision(reason="bf16 v eviction"):
                    for t in range(ntile):
                        nc.vector.tensor_add(
                            vnat[:, sc * ntile + t, :],
                            vps[:, VW * t:VW * (t + 1)], vb_sb)

        # ---------------- RoPE (emitted lazily inside attention) ----------
        # rotate_half is a partition swap within the 32 rot rows: DVE
        # stream_shuffle (per-quadrant permutation) + elementwise combine,
        # all in [32, cols] layout with zero DMAs.  Emitted per (tile,
        # 1024-col slice) just before the attention chunk that needs it.
        SW = 1024                      # rope slice width
        rope_mask = [(i + 16) % 32 for i in range(32)]

        feeder = _Feeder()
        with tc.tile_pool(name="rope", bufs=2) as rp, \
             tc.tile_pool(name="pstrip", bufs=3) as ptp, \
             tc.tile_pool(name="norm", bufs=2) as npool, \
             tc.tile_pool(name="outsb", bufs=4) as op, \
             tc.tile_pool(name="psA", bufs=3, space="PSUM") as psA, \
             tc.tile_pool(name="psY", bufs=2, space="PSUM") as psY, \
             tc.tile_pool(name="psS", bufs=1, space="PSUM") as psS, \
             tc.tile_pool(name="psD", bufs=2, space="PSUM") as psD:

            def rope_slice(i, sl):
                cs = slice(sl * SW, (sl + 1) * SW)
                blk = qk_sb[0:cfg.ROT, i, cs]
                sw = rp.tile([32, SW], BF16, tag="swap", name="sw")
                nc.vector.stream_shuffle(sw, blk, rope_mask)
                with nc.allow_low_precision(reason="bf16 rope"):
                    nc.vector.tensor_mul(sw, sw, sin_sb[:, cs])
                    nc.vector.tensor_mul(blk, blk, cos_sb[:, cs])
                    nc.vector.tensor_add(blk, blk, sw)

            def dense_steps(b, scp):
                # one (eo) output row-tile over two 512-token col chunks
                for eo in range(EO):
                    ot = op.tile([128, 1024], BF16, tag="out")
                    for t in range(2):
                        col = b * S + (2 * scp + t) * 512
                        pd = psD.tile([128, 512], F32, tag="D")
                        for ct in range(CT):
                            nc.tensor.matmul(
                                pd,
                                wd_sb[:, ct, 128 * eo:128 * (eo + 1)],
                                yT_sb[:, ct, col:col + 512],
                                start=(ct == 0), stop=(ct == CT - 1),
                                skip_group_check=True)
                        yield
                        with nc.allow_low_precision(reason="bf16 out"):
                            if (eo + t) % 2 == 0:
                                nc.vector.tensor_copy(
                                    ot[:, 512 * t:512 * (t + 1)], pd)
                            else:
                                nc.scalar.activation(
                                    ot[:, 512 * t:512 * (t + 1)], pd, AF.Copy)
                    nc.sync.dma_start(
                        out=outT[128 * eo:128 * (eo + 1),
                                 b * S + scp * 1024:b * S + (scp + 1) * 1024],
                        in_=ot)
                    yield

            def attention(b, hl):
                scol = b * S
                q_t = qk_sb[:, 2 * hl, scol:scol + S]
                k_t = qk_sb[:, 2 * hl + 1, scol:scol + S]
                pending = [None]   # chunk-end normalization closure

                def emit_chain(c, psYt, psSt):
                    recip = npool.tile([128, 4], F32, tag="recip")
                    nc.vector.reciprocal(recip, psSt[:, 0:4])
                    # transpose each recip column to partition 0 ([1, 128])
                    rps = psA.tile([128, 512], F32, tag="A", name="rps")
                    for g in range(4):
                        nc.tensor.matmul(
                            rps[0:1, 128 * g:128 * (g + 1)],
                            recip[:, g:g + 1], ident,
                            is_transpose=True, start=(g == 0), stop=(g == 3),
                            skip_group_check=True)
                    rT = npool.tile([1, 512], F32, tag="rT")
                    nc.vector.tensor_copy(rT, rps[0:1, 0:512])
                    bc = npool.tile([128, 512], F32, tag="bc")
                    for g in range(4):
                        nc.gpsimd.partition_broadcast(
                            bc[:, 128 * g:128 * (g + 1)],
                            rT[0:1, 128 * g:128 * (g + 1)])
                    with nc.allow_low_precision(reason="bf16 y eviction"):
                        nc.vector.tensor_mul(
                            yT_sb[:, hl, scol + c * 512:scol + (c + 1) * 512],
                            psYt[:, 0:512], bc)

                sl0 = b * S // SW          # rope slices for this batch
                sl_next = [sl0]

                for c in range(cfg.NQC):
                    # rope the k/q columns this chunk consumes
                    need = (b * S + 512 * (c + 1) + SW - 1) // SW
                    while sl_next[0] < need:
                        rope_slice(2 * hl, sl_next[0])
                        rope_slice(2 * hl + 1, sl_next[0])
                        sl_next[0] += 1
                    nj = 4 * (c + 1)
                    psYt = psY.tile([128, 512], F32, tag="Y")
                    psSt = psS.tile([128, 4], F32, tag="S")
                    prev = None

                    def emit_ys(j, pT, off, g0, psYt=psYt, psSt=psSt, c=c,
                                nj=nj):
                        nc.tensor.matmul(
                            psYt[:, off:512],
                            vnat[:, b * (S // 128) + j, 128 * hl:128 * (hl + 1)],
                            pT[:, off:512],
                            start=(j == 0), stop=(j == nj - 1),
                            skip_group_check=True)
                        for g in range(g0, 4):
                            # start only on the very first sums matmul of the
                            # chunk (bank-wide zero region); later columns
                            # accumulate onto pending-zero bytes.
                            nc.tensor.matmul(
                                psSt[:, g:g + 1],
                                pT[:, 128 * g:128 * (g + 1)], ones_col,
                                start=(j == 0 and g == 0),
                                stop=(j == nj - 1 and g == 3),
                                skip_group_check=True)

                    for j in range(nj):
                        g0 = max(0, j - 4 * c)
                        off = 128 * g0
                        ps = psA.tile([128, 512], F32, tag="A")
                        nc.tensor.matmul(
                            ps[:, off:512],
                            k_t[:, 128 * j:128 * (j + 1)],
                            q_t[:, c * 512 + off:(c + 1) * 512],
                            start=True, stop=True, skip_group_check=True)
                        if j >= 4 * c:
                            nc.vector.tensor_add(
                                ps[:, off:off + 128], ps[:, off:off + 128],
                                mask_sb)
                        pT = ptp.tile([128, 512], BF16, tag="p")
                        nc.scalar.activation(
                            pT[:, off:512], ps[:, off:512], AF.Exp,
                            scale=cfg.SCALE)
                        if prev is not None:
                            emit_ys(*prev)
                        if j == 0 and pending[0] is not None:
                            pending[0]()
                            pending[0] = None
                        feeder.step()
                        prev = (j, pT, off, g0)
                    emit_ys(*prev)
                    pending[0] = (lambda c=c, y=psYt, s=psSt:
                                  emit_chain(c, y, s))
                if pending[0] is not None:
                    pending[0]()
                    pending[0] = None

            for b in range(B):
                for hl in range(HPC):
                    attention(b, hl)
                    if hl == HPC - 1:
                        for scp in range(cfg.NQC // 2):
                            feeder.push(dense_steps(b, scp))
            feeder.drain()

            if debug:
                dqk = nc.dram_tensor("dbg_qk", [128, NQK, SF], BF16,
                                     kind="ExternalOutput")
                dv = nc.dram_tensor("dbg_v", [128, NT, VW], BF16,
                                    kind="ExternalOutput")
                dy = nc.dram_tensor("dbg_y", [128, CT, SF], BF16,
                                    kind="ExternalOutput")
                nc.sync.dma_start(out=dqk[:, :, :], in_=qk_sb[:, :, :])
                nc.sync.dma_start(out=dv[:, :, :], in_=vnat[:, :, :])
                nc.sync.dma_start(out=dy[:, :, :], in_=yT_sb[:, :, :])

    nc.finalize()
    return nc


# ---------------------------------------------------------------------------
# Host-side input preparation / sharding
# ---------------------------------------------------------------------------

def _bf16(a):
    import ml_dtypes
    return np.ascontiguousarray(a, np.float32).astype(ml_dtypes.bfloat16)


def _rope_tables(cfg: Cfg):
    inv_freq = 1.0 / (10000.0 ** (np.arange(0, cfg.ROT, 2, dtype=np.float64)
                                  / cfg.ROT))
    t = np.arange(cfg.S, dtype=np.float64)
    freqs = np.outer(t, inv_freq)                       # [S, 16]
    emb = np.concatenate([freqs, freqs], axis=-1)       # [S, 32]
    cos = np.cos(emb).T.astype(np.float32)              # [32, S]
    sin = np.sin(emb).T.astype(np.float32)
    cosF = np.tile(cos, (1, cfg.B))                     # [32, SF]
    sinF = np.tile(sin, (1, cfg.B))
    sinF[:cfg.ROT // 2] *= -1.0                         # fold rotate_half sign
    return _bf16(cosF), _bf16(sinF)


def make_in_maps(cfg: Cfg, x, w_qkv, b_qkv, w_dense):
    HS, HPC = cfg.HS, cfg.HPC
    xTb = _bf16(np.ascontiguousarray(
        np.asarray(x, np.float32).reshape(cfg.SF, cfg.E).T))
    cos128, sin128s = _rope_tables(cfg)
    p = np.arange(128)[:, None]
    f = np.arange(128)[None, :]
    maskT = np.where(p <= f, 0.0, NEG_MASK).astype(np.float32)
    in_maps = []
    for i in range(cfg.n_cores):
        heads = [HPC * i + h for h in range(HPC)]
        qk_rows = np.concatenate(
            [np.arange(h * 3 * HS + qk * HS, h * 3 * HS + (qk + 1) * HS)
             for h in heads for qk in range(2)])
        v_rows = np.concatenate(
            [np.arange(h * 3 * HS + 2 * HS, h * 3 * HS + 3 * HS)
             for h in heads])
        wcat = np.concatenate(
            [np.asarray(w_qkv, np.float32)[qk_rows, :].T,
             np.asarray(w_qkv, np.float32)[v_rows, :].T], axis=1)
        cols = slice(i * cfg.VW, (i + 1) * cfg.VW)
        in_maps.append({
            "xT": xTb,
            "wcat": _bf16(wcat),
            "bqk": np.ascontiguousarray(
                np.asarray(b_qkv, np.float32)[qk_rows]),
            "vbbc": np.ascontiguousarray(np.tile(
                np.asarray(b_qkv, np.float32)[v_rows][None, :], (128, 1))),
            "wdT": _bf16(np.asarray(w_dense, np.float32)[:, cols].T),
            "cosT": cos128,
            "sinT": sin128s,
            "maskT": maskT,
        })
    return in_maps


def combine_outputs(cfg: Cfg, results, b_dense):
    acc = np.zeros((cfg.E, cfg.SF), dtype=np.float64)
    for r in results:
        acc += np.asarray(r["outT"]).astype(np.float64)
    out = acc.T.reshape(cfg.B, cfg.S, cfg.E) + \
        np.asarray(b_dense, np.float64)
    return out.astype(np.float32)


_PROGRAM_CACHE = {}


def kernel(x, w_qkv, b_qkv, w_dense, b_dense):
    from concourse.bass_utils import run_bass_kernel_spmd

    cfg = Cfg()
    key = "full"
    if key not in _PROGRAM_CACHE:
        _PROGRAM_CACHE[key] = build_program(cfg)
    nc = _PROGRAM_CACHE[key]
    in_maps = make_in_maps(cfg, np.asarray(x), np.asarray(w_qkv),
                           np.asarray(b_qkv), np.asarray(w_dense))
    res = run_bass_kernel_spmd(nc, in_maps, list(range(cfg.n_cores)))
    return combine_outputs(cfg, res.results, np.asarray(b_dense))


# revision 32
# speedup vs baseline: 1.5172x; 1.0352x over previous
"""GPT-NeoX attention layer (B=2, S=2048, E=2048, H=16, partial RoPE 32/128)
as a Bass/Tile kernel for 8 Trainium2 NeuronCores.

Sharding: tensor-parallel across heads (2 heads per core, Megatron-style).
Each core computes QKV projection for its 2 heads over all tokens, applies
partial RoPE, runs causal attention, and produces a partial dense output
(contraction over its 256 columns of w_dense).  The 8 bf16 partial outputs
are summed on the host and the dense bias is added once on the host.

Everything on-device is bf16 (inputs pre-converted on the host); PSUM
accumulation stays fp32.  Key structure choices:

  qk_sb  [128, 4, SF]   Q^T/K^T per head (head dim on partitions) - scores
                        and y^T matmuls consume this directly.
  vnat   [128, SF/128, 256]  V in NATURAL [token, d] layout, produced in
                        phase 1 by x-stationary matmuls (x as lhsT), so no
                        PE transposes of V are ever needed.
  scores S^T = K^T.T @ Q^T in [sk, sq] blocks; exp on ScalarE (pipelined one
                        block behind the scores matmuls).
  softmax sums          via N=1 matmuls with the exp'd block as the
                        stationary operand (out [sq,1]): nearly free on PE,
                        instead of a 512-wide ones-matmul per block.
  normalize             reciprocal -> tiny PE transpose -> GPSIMD
                        partition_broadcast -> one DVE multiply per chunk.
  dense                 interleaved into later attention heads (fills the
                        tensor engine while ScalarE works through exp).
"""

import numpy as np
from contextlib import ExitStack

import concourse.bass as bass
import concourse.bacc as bacc
import concourse.mybir as mybir
import concourse.tile as tile
from concourse.masks import make_identity

AF = mybir.ActivationFunctionType
F32 = mybir.dt.float32
BF16 = mybir.dt.bfloat16

NEG_MASK = -1.0e9


class Cfg:
    def __init__(self, B=2, S=2048, E=2048, H=16, n_cores=8):
        self.B, self.S, self.E, self.H = B, S, E, H
        self.HS = 128                  # head size (one partition tile)
        self.ROT = 32                  # rotary dims
        self.n_cores = n_cores
        self.HPC = H // n_cores        # heads per core
        assert self.HPC == 2, "kernel assumes 2 heads per core"
        self.NQK = 2 * self.HPC        # q/k row tiles (h0q,h0k,h1q,h1k)
        self.VW = self.HPC * self.HS   # v natural width (d per core)
        self.RW = self.NQK * self.HS   # per-core q+k rows
        self.WCOLS = self.RW + self.VW
        self.SF = B * S
        self.KT = E // 128             # contraction tiles
        self.SC = 256                  # phase-1 token chunk
        self.NP1 = self.SF // self.SC
        self.G = self.SF // 4          # rope regroup width
        self.NQC = S // 512            # q chunks per (b, h)
        self.EO = E // 128             # dense output row tiles
        self.CT = self.HPC             # dense contraction tiles
        self.SCALE = 1.0 / np.sqrt(self.HS)
        assert S % 512 == 0 and E % 128 == 0 and self.SF % (4 * self.SC) == 0


class _Feeder:
    """Round-robin sink of deferred emission micro-steps (dense tiles)."""

    def __init__(self):
        self.gens = []

    def push(self, gen):
        self.gens.append(gen)

    def step(self):
        while self.gens:
            try:
                next(self.gens[0])
                return
            except StopIteration:
                self.gens.pop(0)

    def drain(self):
        while self.gens:
            g = self.gens.pop(0)
            for _ in g:
                pass


def build_program(cfg: Cfg, debug: bool = False) -> bass.Bass:
    B, S, E = cfg.B, cfg.S, cfg.E
    SF, KT, G = cfg.SF, cfg.KT, cfg.G
    SC, NQK, VW, RW = cfg.SC, cfg.NQK, cfg.VW, cfg.RW
    HPC, CT, EO = cfg.HPC, cfg.CT, cfg.EO
    NT = SF // 128                   # vnat token tiles

    nc = bacc.Bacc(None)
    xT = nc.dram_tensor("xT", [E, SF], BF16, kind="ExternalInput")
    wcat = nc.dram_tensor("wcat", [E, cfg.WCOLS], BF16, kind="ExternalInput")
    bqk = nc.dram_tensor("bqk", [RW], F32, kind="ExternalInput")
    vbbc = nc.dram_tensor("vbbc", [128, VW], F32, kind="ExternalInput")
    wdT = nc.dram_tensor("wdT", [VW, E], BF16, kind="ExternalInput")
    cosT = nc.dram_tensor("cosT", [32, SF], BF16, kind="ExternalInput")
    sinT = nc.dram_tensor("sinT", [32, SF], BF16, kind="ExternalInput")
    maskT = nc.dram_tensor("maskT", [128, 128], BF16, kind="ExternalInput")
    outT = nc.dram_tensor("outT", [E, SF], BF16, kind="ExternalOutput")

    with tile.TileContext(nc) as tc, ExitStack() as stk:
        consts = stk.enter_context(tc.tile_pool(name="consts", bufs=1))
        bigp = stk.enter_context(tc.tile_pool(name="big", bufs=1))
        qk_sb = bigp.tile([128, NQK, SF], BF16)
        vnat = bigp.tile([128, NT, VW], BF16)
        yT_sb = bigp.tile([128, CT, SF], BF16)

        # RoPE: rotate_half is a partition swap within the 32 rot rows ->
        # DVE stream_shuffle (per-quadrant permutation) + elementwise
        # combine in [32, cols] layout, zero DMAs.  Each 1024-col slice is
        # emitted as soon as the phase-1 chunks covering it are done.
        SW = 1024
        rope_mask = [(i + 16) % 32 for i in range(32)]
        ropep = stk.enter_context(tc.tile_pool(name="rope", bufs=2))

        def rope_slice(i, sl):
            cs = slice(sl * SW, (sl + 1) * SW)
            blk = qk_sb[0:cfg.ROT, i, cs]
            sw = ropep.tile([32, SW], BF16, tag="swap", name="sw")
            nc.vector.stream_shuffle(sw, blk, rope_mask)
            with nc.allow_low_precision(reason="bf16 rope"):
                nc.vector.tensor_mul(sw, sw, sin_sb[:, cs])
                nc.vector.tensor_mul(blk, blk, cos_sb[:, cs])
                nc.vector.tensor_add(blk, blk, sw)

        # ---------------- Phase 1: QKV projection ------------------------
        with tc.tile_pool(name="wq", bufs=1) as wp, \
             tc.tile_pool(name="xs", bufs=2) as xp, \
             tc.tile_pool(name="ps1", bufs=2, space="PSUM") as pp1:
            w_sb = wp.tile([128, KT, cfg.WCOLS], BF16)
            w_view = wcat.rearrange("(kt p) r -> p kt r", p=128)
            x_view = xT.rearrange("(kt p) s -> p kt s", p=128)

            # interleave per-kt w loads with quarters of the first x chunk
            xt0 = xp.tile([128, KT, SC], BF16, tag="xt")
            ktg = max(1, KT // 4)
            for q0 in range(0, KT, ktg):
                q1 = min(q0 + ktg, KT)
                for kt in range(q0, q1):
                    nc.sync.dma_start(out=w_sb[:, kt, :], in_=w_view[:, kt, :])
                nc.sync.dma_start(out=xt0[:, q0:q1, :],
                                  in_=x_view[:, q0:q1, 0:SC])
            xt1 = xp.tile([128, KT, SC], BF16, tag="xt")
            nc.sync.dma_start(out=xt1[:, :, :], in_=x_view[:, :, SC:2 * SC])

            # constants (after the critical w/x stream)
            ident = consts.tile([128, 128], F32)
            make_identity(nc, ident)
            identB = consts.tile([128, 128], BF16)
            with nc.allow_low_precision(reason="bf16 identity"):
                nc.vector.tensor_copy(identB, ident)
            ones_tmp = consts.tile([128, 1], BF16, tag="onestmp")
            nc.vector.memset(ones_tmp, 1.0)
            ones_col = ones_tmp
            mask_sb = consts.tile([128, 128], BF16)
            nc.sync.dma_start(out=mask_sb, in_=maskT[:, :])
            bqk_sb = consts.tile([128, NQK], F32)
            nc.sync.dma_start(out=bqk_sb,
                              in_=bqk.rearrange("(rt p) -> p rt", p=128))
            vb_sb = consts.tile([128, VW], F32)
            nc.sync.dma_start(out=vb_sb, in_=vbbc[:, :])
            cos_sb = consts.tile([32, SF], BF16, tag="costab")
            sin_sb = consts.tile([32, SF], BF16, tag="sintab")
            nc.sync.dma_start(out=cos_sb, in_=cosT[:, :])
            nc.sync.dma_start(out=sin_sb, in_=sinT[:, :])
            wd_sb = consts.tile([128, CT, E], BF16, tag="wd")
            nc.sync.dma_start(
                out=wd_sb[:, :, :],
                in_=wdT.rearrange("(ct p) e -> p ct e", p=128))

            ntile = SC // 128   # v token sub-tiles per chunk (=2)
            for sc in range(cfg.NP1):
                if sc == 0:
                    xt = xt0
                elif sc == 1:
                    xt = xt1
                else:
                    xt = xp.tile([128, KT, SC], BF16, tag="xt")
                    nc.sync.dma_start(
                        out=xt[:, :, :],
                        in_=x_view[:, :, sc * SC:(sc + 1) * SC])
                qkps = [pp1.tile([128, 512], F32, tag=f"qk{i}", name=f"qkps{i}")
                        for i in range(NQK // 2)]
                vps = pp1.tile([128, 512], F32, tag="v")
                # NOTE: a start=True matmul marks the PSUM bank's whole 2KB
                # zero-region pending-zero, so only the FIRST matmul into
                # each bank starts; co-resident column groups accumulate
                # onto pending-zero bytes.
                for kt in range(KT):
                    fl, ll = (kt == 0), (kt == KT - 1)
                    for i in range(NQK):
                        nc.tensor.matmul(
                            qkps[i // 2][:, 256 * (i % 2):256 * (i % 2) + SC],
                            w_sb[:, kt, 128 * i:128 * (i + 1)],
                            xt[:, kt, :],
                            start=fl and i % 2 == 0,
                            stop=ll and i % 2 == 1, skip_group_check=True)
                    for t in range(ntile):
                        nc.tensor.matmul(
                            vps[:, VW * t:VW * (t + 1)],
                            xt[:, kt, 128 * t:128 * (t + 1)],
                            w_sb[:, kt, RW:RW + VW],
                            start=fl and t == 0,
                            stop=ll and t == ntile - 1, skip_group_check=True)
                for i in range(NQK):
                    nc.scalar.activation(
                        qk_sb[:, i, sc * SC:(sc + 1) * SC],
                        qkps[i // 2][:, 256 * (i % 2):256 * (i % 2) + SC],
                        AF.Identity, bias=bqk_sb[:, i:i + 1])
                with nc.allow_low_precision(reason="bf16 v eviction"):
                    for t in range(ntile):
                        nc.vector.tensor_add(
                            vnat[:, sc * ntile + t, :],
                            vps[:, VW * t:VW * (t + 1)], vb_sb)
                if (sc + 1) % (SW // SC) == 0:
                    for i in range(NQK):
                        rope_slice(i, sc // (SW // SC))

        # ---------------- Attention + interleaved dense -------------------
        feeder = _Feeder()
        with tc.tile_pool(name="pstrip", bufs=3) as ptp, \
             tc.tile_pool(name="norm", bufs=2) as npool, \
             tc.tile_pool(name="outsb", bufs=4) as op, \
             tc.tile_pool(name="psA", bufs=3, space="PSUM") as psA, \
             tc.tile_pool(name="psY", bufs=2, space="PSUM") as psY, \
             tc.tile_pool(name="psS", bufs=1, space="PSUM") as psS, \
             tc.tile_pool(name="psD", bufs=2, space="PSUM") as psD:

            def dense_steps(b, scp):
                # one (eo) output row-tile over two 512-token col chunks
                for eo in range(EO):
                    ot = op.tile([128, 1024], BF16, tag="out")
                    for t in range(2):
                        col = b * S + (2 * scp + t) * 512
                        pd = psD.tile([128, 512], F32, tag="D")
                        for ct in range(CT):
                            nc.tensor.matmul(
                                pd,
                                wd_sb[:, ct, 128 * eo:128 * (eo + 1)],
                                yT_sb[:, ct, col:col + 512],
                                start=(ct == 0), stop=(ct == CT - 1),
                                skip_group_check=True)
                        yield
                        with nc.allow_low_precision(reason="bf16 out"):
                            if (eo + t) % 2 == 0:
                                nc.vector.tensor_copy(
                                    ot[:, 512 * t:512 * (t + 1)], pd)
                            else:
                                nc.scalar.activation(
                                    ot[:, 512 * t:512 * (t + 1)], pd, AF.Copy)
                    nc.sync.dma_start(
                        out=outT[128 * eo:128 * (eo + 1),
                                 b * S + scp * 1024:b * S + (scp + 1) * 1024],
                        in_=ot)
                    yield

            def attention(b, hl, on_chain=None):
                scol = b * S
                q_t = qk_sb[:, 2 * hl, scol:scol + S]
                k_t = qk_sb[:, 2 * hl + 1, scol:scol + S]
                pending = [None]   # chunk-end normalization closure

                def emit_chain(c, psYt, psSt):
                    recip = npool.tile([128, 4], F32, tag="recip")
                    nc.vector.reciprocal(recip, psSt[:, 0:4])
                    # transpose each recip column to partition 0 ([1, 128])
                    rps = psA.tile([128, 512], F32, tag="A", name="rps")
                    for g in range(4):
                        nc.tensor.matmul(
                            rps[0:1, 128 * g:128 * (g + 1)],
                            recip[:, g:g + 1], ident,
                            is_transpose=True, start=(g == 0), stop=(g == 3),
                            skip_group_check=True)
                    rT = npool.tile([1, 512], F32, tag="rT")
                    nc.vector.tensor_copy(rT, rps[0:1, 0:512])
                    bc = npool.tile([128, 512], F32, tag="bc")
                    for g in range(4):
                        nc.gpsimd.partition_broadcast(
                            bc[:, 128 * g:128 * (g + 1)],
                            rT[0:1, 128 * g:128 * (g + 1)])
                    with nc.allow_low_precision(reason="bf16 y eviction"):
                        nc.vector.tensor_mul(
                            yT_sb[:, hl, scol + c * 512:scol + (c + 1) * 512],
                            psYt[:, 0:512], bc)
                    if on_chain is not None:
                        on_chain(c)

                for c in range(cfg.NQC):
                    nj = 4 * (c + 1)
                    psYt = psY.tile([128, 512], F32, tag="Y")
                    psSt = psS.tile([128, 4], F32, tag="S")
                    prev = None

                    def emit_ys(j, pT, off, g0, psYt=psYt, psSt=psSt, c=c,
                                nj=nj):
                        nc.tensor.matmul(
                            psYt[:, off:512],
                            vnat[:, b * (S // 128) + j, 128 * hl:128 * (hl + 1)],
                            pT[:, off:512],
                            start=(j == 0), stop=(j == nj - 1),
                            skip_group_check=True)
                        for g in range(g0, 4):
                            # start only on the very first sums matmul of the
                            # chunk (bank-wide zero region); later columns
                            # accumulate onto pending-zero bytes.
                            nc.tensor.matmul(
                                psSt[:, g:g + 1],
                                pT[:, 128 * g:128 * (g + 1)], ones_col,
                                start=(j == 0 and g == 0),
                                stop=(j == nj - 1 and g == 3),
                                skip_group_check=True)

                    for j in range(nj):
                        g0 = max(0, j - 4 * c)
                        off = 128 * g0
                        ps = psA.tile([128, 512], F32, tag="A")
                        nc.tensor.matmul(
                            ps[:, off:512],
                            k_t[:, 128 * j:128 * (j + 1)],
                            q_t[:, c * 512 + off:(c + 1) * 512],
                            start=True, stop=True, skip_group_check=True)
                        pT = ptp.tile([128, 512], BF16, tag="p")
                        nc.scalar.activation(
                            pT[:, off:512], ps[:, off:512], AF.Exp,
                            scale=cfg.SCALE)
                        if j >= 4 * c:
                            # causal mask as a cheap post-exp 0/1 multiply
                            with nc.allow_low_precision(reason="bf16 mask"):
                                nc.vector.tensor_mul(
                                    pT[:, off:off + 128],
                                    pT[:, off:off + 128], mask_sb)
                        if prev is not None:
                            emit_ys(*prev)
                        if j == 0 and pending[0] is not None:
                            pending[0]()
                            pending[0] = None
                        feeder.step()
                        prev = (j, pT, off, g0)
                    emit_ys(*prev)
                    feeder.step()
                    pending[0] = (lambda c=c, y=psYt, s=psSt:
                                  emit_chain(c, y, s))
                if pending[0] is not None:
                    pending[0]()
                    pending[0] = None

            for b in range(B):
                for hl in range(HPC):
                    if hl == HPC - 1:
                        # dense cols [0:1024*(scp+1)) ready once this head's
                        # chunk 2*scp+1 is normalized
                        hook = (lambda c, b=b: feeder.push(
                            dense_steps(b, (c - 1) // 2)) if c % 2 == 1
                            else None)
                    else:
                        hook = None
                    attention(b, hl, on_chain=hook)
            feeder.drain()

            if debug:
                dqk = nc.dram_tensor("dbg_qk", [128, NQK, SF], BF16,
                                     kind="ExternalOutput")
                dv = nc.dram_tensor("dbg_v", [128, NT, VW], BF16,
                                    kind="ExternalOutput")
                dy = nc.dram_tensor("dbg_y", [128, CT, SF], BF16,
                                    kind="ExternalOutput")
                nc.sync.dma_start(out=dqk[:, :, :], in_=qk_sb[:, :, :])
                nc.sync.dma_start(out=dv[:, :, :], in_=vnat[:, :, :])
                nc.sync.dma_start(out=dy[:, :, :], in_=yT_sb[:, :, :])

    nc.finalize()
    return nc


# ---------------------------------------------------------------------------
# Host-side input preparation / sharding
# ---------------------------------------------------------------------------

def _bf16(a):
    import ml_dtypes
    return np.ascontiguousarray(a, np.float32).astype(ml_dtypes.bfloat16)


def _rope_tables(cfg: Cfg):
    inv_freq = 1.0 / (10000.0 ** (np.arange(0, cfg.ROT, 2, dtype=np.float64)
                                  / cfg.ROT))
    t = np.arange(cfg.S, dtype=np.float64)
    freqs = np.outer(t, inv_freq)                       # [S, 16]
    emb = np.concatenate([freqs, freqs], axis=-1)       # [S, 32]
    cos = np.cos(emb).T.astype(np.float32)              # [32, S]
    sin = np.sin(emb).T.astype(np.float32)
    cosF = np.tile(cos, (1, cfg.B))                     # [32, SF]
    sinF = np.tile(sin, (1, cfg.B))
    sinF[:cfg.ROT // 2] *= -1.0                         # fold rotate_half sign
    return _bf16(cosF), _bf16(sinF)


def make_in_maps(cfg: Cfg, x, w_qkv, b_qkv, w_dense):
    HS, HPC = cfg.HS, cfg.HPC
    xTb = _bf16(np.ascontiguousarray(
        np.asarray(x, np.float32).reshape(cfg.SF, cfg.E).T))
    cos128, sin128s = _rope_tables(cfg)
    p = np.arange(128)[:, None]
    f = np.arange(128)[None, :]
    maskT = _bf16(np.where(p <= f, 1.0, 0.0))   # post-exp 0/1 causal mask
    in_maps = []
    for i in range(cfg.n_cores):
        heads = [HPC * i + h for h in range(HPC)]
        qk_rows = np.concatenate(
            [np.arange(h * 3 * HS + qk * HS, h * 3 * HS + (qk + 1) * HS)
             for h in heads for qk in range(2)])
        v_rows = np.concatenate(
            [np.arange(h * 3 * HS + 2 * HS, h * 3 * HS + 3 * HS)
             for h in heads])
        wcat = np.concatenate(
            [np.asarray(w_qkv, np.float32)[qk_rows, :].T,
             np.asarray(w_qkv, np.float32)[v_rows, :].T], axis=1)
        cols = slice(i * cfg.VW, (i + 1) * cfg.VW)
        in_maps.append({
            "xT": xTb,
            "wcat": _bf16(wcat),
            "bqk": np.ascontiguousarray(
                np.asarray(b_qkv, np.float32)[qk_rows]),
            "vbbc": np.ascontiguousarray(np.tile(
                np.asarray(b_qkv, np.float32)[v_rows][None, :], (128, 1))),
            "wdT": _bf16(np.asarray(w_dense, np.float32)[:, cols].T),
            "cosT": cos128,
            "sinT": sin128s,
            "maskT": maskT,
        })
    return in_maps


def combine_outputs(cfg: Cfg, results, b_dense):
    acc = np.zeros((cfg.E, cfg.SF), dtype=np.float64)
    for r in results:
        acc += np.asarray(r["outT"]).astype(np.float64)
    out = acc.T.reshape(cfg.B, cfg.S, cfg.E) + \
        np.asarray(b_dense, np.float64)
    return out.astype(np.float32)


_PROGRAM_CACHE = {}


def kernel(x, w_qkv, b_qkv, w_dense, b_dense):
    from concourse.bass_utils import run_bass_kernel_spmd

    cfg = Cfg()
    key = "full"
    if key not in _PROGRAM_CACHE:
        _PROGRAM_CACHE[key] = build_program(cfg)
    nc = _PROGRAM_CACHE[key]
    in_maps = make_in_maps(cfg, np.asarray(x), np.asarray(w_qkv),
                           np.asarray(b_qkv), np.asarray(w_dense))
    res = run_bass_kernel_spmd(nc, in_maps, list(range(cfg.n_cores)))
    return combine_outputs(cfg, res.results, np.asarray(b_dense))
